# revision 17
# baseline (speedup 1.0000x reference)
"""Trainium2 Bass kernel for nn_Dense_1322849927863 (segment_reduce).

Reference computation:
  h   = einsum('bltf,l->btf', x, aggr_w)            # layer aggregation (L=12)
  h   = relu(h @ w1.T + b1)                         # [B,T,H=256]
  h   = relu(h @ w2.T + b2)                         # [B,T,256]
  pooled = (h * mask).sum(t) / lengths              # masked mean over t<len
  out = pooled @ w3.T + b3                          # [B,8]

Strategy (8 NeuronCores, data parallel over batch):
  - Host pairs the 16 batches (longest+shortest valid length) to balance
    per-core work and packs ONLY the valid t-rows of each pair into a dense
    buffer per core (masked rows never influence the output).  The packed
    buffer is laid out as xq[gt, l, g, f] with t = 10*g + gt so that one
    120-partition, 2-dim-AP DMA (36 KiB contiguous per partition) loads a
    full 120-t-row sub-tile as [partition=(gt,l), free=(g,f)].
  - Layer aggregation = 12 accumulating float32r matmuls per sub-tile with
    shifted block-diagonal stationary matrices -> hagg[t,f] in PSUM
    (float32r moves 1 column/cycle at N>=256; plain fp32 needs 4).
  - TensorE transposes flip hagg to [f,t]; two matmul chains apply w1/w2
    with fused bias+relu on ScalarE; masked pooling = DVE multiply +
    free-axis reduce with host-prepared (t<len)/len masks per segment slot.
    The 8-way classifier runs on-chip; host just reorders [2,8] per core.
"""

import numpy as np

B, L, T, F = 16, 12, 1024, 768
H, NL = 256, 8
NCORES = 8
P = 128
G = 10           # t-positions per aggregation group
SUB = 120        # t-rows per sub-tile (12 groups of 10), K = 120
FC = 384         # f columns per aggregation PSUM tile (2 chunks = 768)

_CACHE = {}
LAST_RESULTS = None  # BassKernelResults from the most recent run (for test.py)


def _macro_split(ns):
    """Group sub-tiles into macro tiles of >=2 where possible (N>=256 keeps
    float32r at full speed; a single short tail macro is negligible)."""
    macros = []
    s = 0
    while ns - s > 4:
        macros.append((s, 3))
        s += 3
    if ns - s == 4:
        macros.extend([(s, 2), (s + 2, 2)])
    elif ns - s > 0:
        macros.append((s, ns - s))
    return macros


def _build_bass(tpad, dbg=False):
    import concourse.bass as bass
    import concourse.mybir as mybir
    import concourse.tile as tile
    from concourse import bacc

    f32 = mybir.dt.float32
    f32r = mybir.dt.float32r
    AF = mybir.ActivationFunctionType
    AX = mybir.AxisListType

    ns = tpad // SUB
    nt10 = tpad // G
    macros = _macro_split(ns)
    nmac = len(macros)

    nc = bacc.Bacc()
    x_h = nc.dram_tensor("x", [G, L, nt10, F], f32r, kind="ExternalInput")
    mk_h = nc.dram_tensor("masks", [2, tpad], f32, kind="ExternalInput")
    ag_h = nc.dram_tensor("aggw", [12, SUB, SUB], f32r, kind="ExternalInput")
    w1_h = nc.dram_tensor("w1t", [P, 6, H], f32r, kind="ExternalInput")
    w2_h = nc.dram_tensor("w2t", [P, 2, H], f32r, kind="ExternalInput")
    b1_h = nc.dram_tensor("b1s", [P, 2], f32, kind="ExternalInput")
    b2_h = nc.dram_tensor("b2s", [P, 2], f32, kind="ExternalInput")
    w3_h = nc.dram_tensor("w3t", [P, 2, NL], f32, kind="ExternalInput")
    b3_h = nc.dram_tensor("b3s", [NL, 1], f32, kind="ExternalInput")
    id_h = nc.dram_tensor("ident", [SUB, SUB], f32r, kind="ExternalInput")
    out_h = nc.dram_tensor("out", [2, NL], f32, kind="ExternalOutput")
    if dbg:
        dbg_hagg = nc.dram_tensor("dbg_hagg", [SUB, F], f32r, kind="ExternalOutput")
        dbg_h3 = nc.dram_tensor("dbg_h3", [P, 2, 3 * SUB], f32, kind="ExternalOutput")
        dbg_mask = nc.dram_tensor("dbg_mask", [P, 2, tpad], f32, kind="ExternalOutput")
        dbg_pool = nc.dram_tensor("dbg_pool", [P, 2, 2], f32, kind="ExternalOutput")
        nmac_ = len(_macro_split(tpad // SUB))
        dbg_acc = nc.dram_tensor("dbg_acc", [P, 2, 2, nmac_], f32, kind="ExternalOutput")
        dbg_h3b = nc.dram_tensor("dbg_h3b", [P, 2, 3 * SUB], f32, kind="ExternalOutput")

    with tile.TileContext(nc) as tc:
        with (
            tc.tile_pool(name="const", bufs=1) as const,
            tc.tile_pool(name="xp", bufs=2) as xp,
            tc.tile_pool(name="hp", bufs=2) as hp,
            tc.tile_pool(name="tp", bufs=2) as tp,
            tc.tile_pool(name="fin", bufs=2) as fin,
            tc.tile_pool(name="psA", bufs=2, space="PSUM") as psA,
            tc.tile_pool(name="psT", bufs=1, space="PSUM") as psT,
            tc.tile_pool(name="ps1", bufs=1, space="PSUM") as ps1,
            tc.tile_pool(name="ps2", bufs=1, space="PSUM") as ps2,
            tc.tile_pool(name="ps3", bufs=1, space="PSUM") as ps3,
        ):
            # ---- constants into SBUF ----
            ag_sb = const.tile([SUB, 12, SUB], f32r)
            nc.sync.dma_start(
                out=ag_sb,
                in_=bass.AP(ag_h, 0, [[SUB, SUB], [SUB * SUB, 12], [1, SUB]]),
            )
            w1_sb = const.tile([P, 6, H], f32r)
            nc.sync.dma_start(out=w1_sb, in_=w1_h[:, :, :])
            w2_sb = const.tile([P, 2, H], f32r)
            nc.sync.dma_start(out=w2_sb, in_=w2_h[:, :, :])
            b1_sb = const.tile([P, 2], f32)
            nc.sync.dma_start(out=b1_sb, in_=b1_h[:, :])
            b2_sb = const.tile([P, 2], f32)
            nc.sync.dma_start(out=b2_sb, in_=b2_h[:, :])
            w3_sb = const.tile([P, 2, NL], f32)
            nc.sync.dma_start(out=w3_sb, in_=w3_h[:, :, :])
            b3_sb = const.tile([NL, 1], f32)
            nc.sync.dma_start(out=b3_sb, in_=b3_h[:, :])
            id_sb = const.tile([SUB, SUB], f32r)
            nc.sync.dma_start(out=id_sb, in_=id_h[:, :])
            mk_sb = const.tile([P, 2, tpad], f32)
            for s in range(2):
                nc.gpsimd.dma_start(
                    out=mk_sb[:, s, :],
                    in_=bass.AP(mk_h, s * tpad, [[0, P], [1, tpad]]),
                )
            acc_sb = const.tile([P, 2, 2, nmac], f32)

            # ---- main loop over macro tiles ----
            for mi, (s0, msubs) in enumerate(macros):
                W = msubs * SUB
                haggT = tp.tile([P, 6, 3 * SUB], f32r, tag="haggT")
                for sl in range(msubs):
                    st = s0 + sl
                    x_sb = xp.tile([SUB, 12 * F], f32r, tag="x")
                    nc.sync.dma_start(
                        out=x_sb,
                        in_=bass.AP(
                            x_h, 12 * st * F, [[nt10 * F, SUB], [1, 12 * F]]
                        ),
                    )
                    for fc in range(2):
                        agg_ps = psA.tile([SUB, FC], f32, tag="agg")
                        for i in range(12):
                            nc.tensor.matmul(
                                agg_ps,
                                lhsT=ag_sb[:, i, :],
                                rhs=x_sb[:, i * F + fc * FC:
                                         i * F + (fc + 1) * FC],
                                start=(i == 0),
                                stop=(i == 11),
                            )
                        hagg = hp.tile([SUB, FC], f32r, tag="hagg")
                        nc.scalar.copy(out=hagg, in_=agg_ps)
                        if dbg and st == 0:
                            nc.sync.dma_start(
                                out=bass.AP(dbg_hagg, fc * FC,
                                            [[F, SUB], [1, FC]]),
                                in_=hagg,
                            )
                        tr_ps = psT.tile([P, 3, SUB], f32r, tag="tr")
                        for j in range(3):
                            nc.tensor.transpose(
                                tr_ps[:, j, :],
                                hagg[:, j * P:(j + 1) * P],
                                id_sb,
                            )
                        nc.vector.tensor_copy(
                            out=haggT[:, fc * 3:(fc + 1) * 3,
                                      sl * SUB:(sl + 1) * SUB],
                            in_=tr_ps,
                        )
                # ---- w1 matmul + bias + relu ----
                mm1_ps = ps1.tile([P, 2, 512], f32, tag="mm1")
                for mh in range(2):
                    for kf in range(6):
                        nc.tensor.matmul(
                            mm1_ps[:, mh, :W],
                            lhsT=w1_sb[:, kf, mh * P:(mh + 1) * P],
                            rhs=haggT[:, kf, :W],
                            start=(kf == 0),
                            stop=(kf == 5),
                        )
                h2 = hp.tile([P, 2, 3 * SUB], f32r, tag="h2")
                for mh in range(2):
                    nc.scalar.activation(
                        out=h2[:, mh, :W],
                        in_=mm1_ps[:, mh, :W],
                        func=AF.Relu,
                        bias=b1_sb[:, mh:mh + 1],
                        scale=1.0,
                    )
                # ---- w2 matmul + bias + relu ----
                mm2_ps = ps2.tile([P, 2, 512], f32, tag="mm2")
                for mg in range(2):
                    for kh in range(2):
                        nc.tensor.matmul(
                            mm2_ps[:, mg, :W],
                            lhsT=w2_sb[:, kh, mg * P:(mg + 1) * P],
                            rhs=h2[:, kh, :W],
                            start=(kh == 0),
                            stop=(kh == 1),
                        )
                h3 = hp.tile([P, 2, 3 * SUB], f32, tag="h3")
                for mg in range(2):
                    nc.scalar.activation(
                        out=h3[:, mg, :W],
                        in_=mm2_ps[:, mg, :W],
                        func=AF.Relu,
                        bias=b2_sb[:, mg:mg + 1],
                        scale=1.0,
                    )
                if dbg and mi == 0:
                    nc.sync.dma_start(out=dbg_h3[:, :, :W], in_=h3[:, :, :W])
                if dbg and mi == 1:
                    nc.sync.dma_start(out=dbg_h3b[:, :, :W], in_=h3[:, :, :W])
                # ---- masked pooling (both segment slots) ----
                for s in range(2):
                    h3m = hp.tile([P, 2, 3 * SUB], f32, tag="h3m")
                    for mg in range(2):
                        nc.vector.tensor_mul(
                            out=h3m[:, mg, :W],
                            in0=h3[:, mg, :W],
                            in1=mk_sb[:, s, s0 * SUB:s0 * SUB + W],
                        )
                    nc.vector.reduce_sum(
                        out=acc_sb[:, s, :, mi],
                        in_=h3m[:, :, :W],
                        axis=AX.X,
                    )

            # ---- finale: reduce accumulators, classifier, write out ----
            if dbg:
                nc.sync.dma_start(out=dbg_mask[:, :, :], in_=mk_sb)
                nc.sync.dma_start(out=dbg_acc[:, :, :, :], in_=acc_sb)
            for s in range(2):
                pooled = fin.tile([P, 2], f32, tag="pooled")
                for kg in range(2):
                    nc.vector.reduce_sum(
                        out=pooled[:, kg:kg + 1],
                        in_=acc_sb[:, s, kg, :],
                        axis=AX.X,
                    )
                if dbg:
                    nc.sync.dma_start(
                        out=bass.AP(dbg_pool, s * 2, [[4, P], [1, 2]]),
                        in_=pooled,
                    )
                mm3_ps = ps3.tile([NL, 1], f32, tag="mm3")
                for kg in range(2):
                    nc.tensor.matmul(
                        mm3_ps,
                        lhsT=w3_sb[:, kg, :],
                        rhs=pooled[:, kg:kg + 1],
                        start=(kg == 0),
                        stop=(kg == 1),
                    )
                o_sb = fin.tile([NL, 1], f32, tag="osb")
                nc.scalar.add(out=o_sb, in_=mm3_ps, add=b3_sb)
                nc.sync.dma_start(
                    out=bass.AP(out_h, s * NL, [[1, NL]]),
                    in_=o_sb,
                )
    nc.compile()
    return nc


def _prep_shared(aggr_w, w1, b1, w2, b2, w3, b3):
    aggw = np.zeros((12, SUB, SUB), dtype=np.float32)
    for i in range(12):
        for gt in range(G):
            for l in range(L):
                aggw[i, gt * L + l, i * G + gt] = aggr_w[l]
    w1t = np.ascontiguousarray(
        w1.T.reshape(6, P, H).transpose(1, 0, 2)).astype(np.float32)
    w2t = np.ascontiguousarray(
        w2.T.reshape(2, P, H).transpose(1, 0, 2)).astype(np.float32)
    w3t = np.ascontiguousarray(
        w3.T.reshape(2, P, NL).transpose(1, 0, 2)).astype(np.float32)
    b1s = np.ascontiguousarray(b1.reshape(2, P).T).astype(np.float32)
    b2s = np.ascontiguousarray(b2.reshape(2, P).T).astype(np.float32)
    b3s = b3.reshape(NL, 1).astype(np.float32)
    ident = np.eye(SUB, dtype=np.float32)
    return {
        "aggw": aggw, "w1t": w1t, "w2t": w2t, "b1s": b1s, "b2s": b2s,
        "w3t": w3t, "b3s": b3s, "ident": ident,
    }


def kernel(x, lengths, aggr_w, w1, b1, w2, b2, w3, b3):
    global LAST_RESULTS
    from concourse.bass_utils import run_bass_kernel_spmd

    x = np.asarray(x, dtype=np.float32)
    lens = np.asarray(lengths).astype(np.int64)
    aggr_w = np.asarray(aggr_w, dtype=np.float32)
    w1 = np.asarray(w1, dtype=np.float32)
    b1 = np.asarray(b1, dtype=np.float32)
    w2 = np.asarray(w2, dtype=np.float32)
    b2 = np.asarray(b2, dtype=np.float32)
    w3 = np.asarray(w3, dtype=np.float32)
    b3 = np.asarray(b3, dtype=np.float32)

    # pair longest with shortest to balance per-core work
    order = np.argsort(-lens, kind="stable")
    pairs = [(int(order[i]), int(order[B - 1 - i])) for i in range(NCORES)]
    psum_max = max(int(lens[a] + lens[b]) for a, b in pairs)
    tpad = max(SUB, ((psum_max + SUB - 1) // SUB) * SUB)
    nt10 = tpad // G

    if tpad not in _CACHE:
        _CACHE[tpad] = _build_bass(tpad)
    nc = _CACHE[tpad]

    shared = _prep_shared(aggr_w, w1, b1, w2, b2, w3, b3)
    in_maps = []
    for a, b in pairs:
        la, lb = int(lens[a]), int(lens[b])
        xt = np.zeros((L, tpad, F), dtype=np.float32)
        xt[:, :la] = x[a, :, :la]
        xt[:, la:la + lb] = x[b, :, :lb]
        # xq[gt, l, g, f] = xt[l, 10*g + gt, f]
        xq = np.ascontiguousarray(
            xt.reshape(L, nt10, G, F).transpose(2, 0, 1, 3))
        masks = np.zeros((2, tpad), dtype=np.float32)
        masks[0, :la] = 1.0 / la
        masks[1, la:la + lb] = 1.0 / lb
        in_maps.append({"x": xq, "masks": masks, **shared})

    res = run_bass_kernel_spmd(nc, in_maps, core_ids=list(range(NCORES)))
    LAST_RESULTS = res

    out = np.zeros((B, NL), dtype=np.float32)
    for c, (a, b) in enumerate(pairs):
        out[a] = res.results[c]["out"][0]
        out[b] = res.results[c]["out"][1]
    return out


# revision 20
# speedup vs baseline: 18570.9369x; 18570.9369x over previous
"""Trainium2 Bass kernel for nn_Dense_1322849927863 (segment_reduce).

Reference computation:
  h   = einsum('bltf,l->btf', x, aggr_w)            # layer aggregation (L=12)
  h   = relu(h @ w1.T + b1)                         # [B,T,H=256]
  h   = relu(h @ w2.T + b2)                         # [B,T,256]
  pooled = (h * mask).sum(t) / lengths              # masked mean over t<len
  out = pooled @ w3.T + b3                          # [B,8]

Strategy (8 NeuronCores, data parallel over batch):
  - Host pairs the 16 batches (longest+shortest valid length) to balance
    per-core work and packs ONLY the valid t-rows of each pair into a dense
    buffer per core (masked rows never influence the output).  The packed
    buffer is laid out as xq[gt, l, g, f] with t = 10*g + gt so that one
    120-partition, 2-dim-AP DMA (36 KiB contiguous per partition) loads a
    full 120-t-row sub-tile as [partition=(gt,l), free=(g,f)].
  - Layer aggregation = 12 accumulating float32r matmuls per sub-tile with
    shifted block-diagonal stationary matrices -> hagg[t,f] in PSUM
    (float32r moves 1 column/cycle at N>=256; plain fp32 needs 4).
  - TensorE transposes flip hagg to [f,t]; two matmul chains apply w1/w2
    with fused bias+relu on ScalarE; masked pooling = DVE multiply +
    free-axis reduce with host-prepared (t<len)/len masks per segment slot.
    The 8-way classifier runs on-chip; host just reorders [2,8] per core.
"""

import numpy as np

B, L, T, F = 16, 12, 1024, 768
H, NL = 256, 8
NCORES = 8
P = 128
G = 10           # t-positions per aggregation group
SUB = 120        # t-rows per sub-tile (12 groups of 10), K = 120
FC = 384         # f columns per aggregation PSUM tile (2 chunks = 768)

_CACHE = {}
LAST_RESULTS = None  # BassKernelResults from the most recent run (for test.py)


def _macro_split(ns):
    """Group sub-tiles into macro tiles of >=2 where possible (N>=256 keeps
    float32r at full speed; a single short tail macro is negligible)."""
    macros = []
    s = 0
    while ns - s > 4:
        macros.append((s, 3))
        s += 3
    if ns - s == 4:
        macros.extend([(s, 2), (s + 2, 2)])
    elif ns - s > 0:
        macros.append((s, ns - s))
    return macros


def _build_bass(tpad, dbg=False, reps=0):
    import concourse.bass as bass
    import concourse.mybir as mybir
    import concourse.tile as tile
    from concourse import bacc

    f32 = mybir.dt.float32
    f32r = mybir.dt.float32r
    AF = mybir.ActivationFunctionType
    AX = mybir.AxisListType

    ns = tpad // SUB
    nt10 = tpad // G
    macros = _macro_split(ns)
    nmac = len(macros)

    nc = bacc.Bacc()
    x_h = nc.dram_tensor("x", [G, L, nt10, F], f32r, kind="ExternalInput")
    mk_h = nc.dram_tensor("masks", [2, tpad], f32, kind="ExternalInput")
    ag_h = nc.dram_tensor("aggw", [12, SUB, SUB], f32r, kind="ExternalInput")
    w1_h = nc.dram_tensor("w1t", [P, 6, H], f32r, kind="ExternalInput")
    w2_h = nc.dram_tensor("w2t", [P, 2, H], f32r, kind="ExternalInput")
    b1_h = nc.dram_tensor("b1s", [P, 2], f32, kind="ExternalInput")
    b2_h = nc.dram_tensor("b2s", [P, 2], f32, kind="ExternalInput")
    w3_h = nc.dram_tensor("w3t", [P, 2, NL], f32, kind="ExternalInput")
    b3_h = nc.dram_tensor("b3s", [NL, 1], f32, kind="ExternalInput")
    id_h = nc.dram_tensor("ident", [SUB, SUB], f32r, kind="ExternalInput")
    out_h = nc.dram_tensor("out", [2, NL], f32, kind="ExternalOutput")
    if dbg:
        dbg_hagg = nc.dram_tensor("dbg_hagg", [SUB, F], f32r, kind="ExternalOutput")
        dbg_h3 = nc.dram_tensor("dbg_h3", [P, 2, 3 * SUB], f32, kind="ExternalOutput")
        dbg_mask = nc.dram_tensor("dbg_mask", [P, 2, tpad], f32, kind="ExternalOutput")
        dbg_pool = nc.dram_tensor("dbg_pool", [P, 2, 2], f32, kind="ExternalOutput")
        nmac_ = len(_macro_split(tpad // SUB))
        dbg_acc = nc.dram_tensor("dbg_acc", [P, 2, 2, nmac_], f32, kind="ExternalOutput")
        dbg_h3b = nc.dram_tensor("dbg_h3b", [P, 2, 3 * SUB], f32, kind="ExternalOutput")

    with tile.TileContext(nc) as tc:
        with (
            tc.tile_pool(name="const", bufs=1) as const,
            tc.tile_pool(name="xp", bufs=2) as xp,
            tc.tile_pool(name="hp", bufs=2) as hp,
            tc.tile_pool(name="tp", bufs=2) as tp,
            tc.tile_pool(name="fin", bufs=2) as fin,
            tc.tile_pool(name="psA", bufs=2, space="PSUM") as psA,
            tc.tile_pool(name="psT", bufs=1, space="PSUM") as psT,
            tc.tile_pool(name="ps1", bufs=1, space="PSUM") as ps1,
            tc.tile_pool(name="ps2", bufs=1, space="PSUM") as ps2,
            tc.tile_pool(name="ps3", bufs=1, space="PSUM") as ps3,
        ):
            # ---- constants into SBUF ----
            ag_sb = const.tile([SUB, 12, SUB], f32r)
            nc.sync.dma_start(
                out=ag_sb,
                in_=bass.AP(ag_h, 0, [[SUB, SUB], [SUB * SUB, 12], [1, SUB]]),
            )
            w1_sb = const.tile([P, 6, H], f32r)
            nc.sync.dma_start(out=w1_sb, in_=w1_h[:, :, :])
            w2_sb = const.tile([P, 2, H], f32r)
            nc.sync.dma_start(out=w2_sb, in_=w2_h[:, :, :])
            b1_sb = const.tile([P, 2], f32)
            nc.sync.dma_start(out=b1_sb, in_=b1_h[:, :])
            b2_sb = const.tile([P, 2], f32)
            nc.sync.dma_start(out=b2_sb, in_=b2_h[:, :])
            w3_sb = const.tile([P, 2, NL], f32)
            nc.sync.dma_start(out=w3_sb, in_=w3_h[:, :, :])
            b3_sb = const.tile([NL, 1], f32)
            nc.sync.dma_start(out=b3_sb, in_=b3_h[:, :])
            id_sb = const.tile([SUB, SUB], f32r)
            nc.sync.dma_start(out=id_sb, in_=id_h[:, :])
            mk_sb = const.tile([P, 2, tpad], f32)
            for s in range(2):
                nc.gpsimd.dma_start(
                    out=mk_sb[:, s, :],
                    in_=bass.AP(mk_h, s * tpad, [[0, P], [1, tpad]]),
                )
            acc_sb = const.tile([P, 2, 2, nmac], f32)

            import contextlib
            rep_ctx = tc.For_i(0, reps, 1) if reps else contextlib.nullcontext()
            with rep_ctx:
                _emit_body(nc, tc, bass, mybir, tpad, macros, dbg,
                           locals())
    nc.compile()
    return nc


def _emit_body(nc, tc, bass, mybir, tpad, macros, dbg, env):
    f32 = mybir.dt.float32
    f32r = mybir.dt.float32r
    AF = mybir.ActivationFunctionType
    AX = mybir.AxisListType
    nt10 = tpad // G
    (const, xp, hp, tp, fin, psA, psT, ps1, ps2, ps3) = (
        env[k] for k in
        ("const", "xp", "hp", "tp", "fin", "psA", "psT", "ps1", "ps2", "ps3"))
    ag_sb, w1_sb, w2_sb, b1_sb, b2_sb, w3_sb, b3_sb, id_sb, mk_sb, acc_sb = (
        env[k] for k in ("ag_sb", "w1_sb", "w2_sb", "b1_sb", "b2_sb",
                         "w3_sb", "b3_sb", "id_sb", "mk_sb", "acc_sb"))
    x_h, out_h = env["x_h"], env["out_h"]
    if dbg:
        dbg_hagg, dbg_h3, dbg_mask, dbg_pool = (
            env[k] for k in ("dbg_hagg", "dbg_h3", "dbg_mask", "dbg_pool"))
        dbg_acc, dbg_h3b = env["dbg_acc"], env["dbg_h3b"]
    if True:
        if True:
            # ---- main loop over macro tiles ----
            for mi, (s0, msubs) in enumerate(macros):
                W = msubs * SUB
                haggT = tp.tile([P, 6, 3 * SUB], f32r, tag="haggT")
                for sl in range(msubs):
                    st = s0 + sl
                    x_sb = xp.tile([SUB, 12 * F], f32r, tag="x")
                    nc.sync.dma_start(
                        out=x_sb,
                        in_=bass.AP(
                            x_h, 12 * st * F, [[nt10 * F, SUB], [1, 12 * F]]
                        ),
                    )
                    for fc in range(2):
                        agg_ps = psA.tile([SUB, FC], f32, tag="agg")
                        for i in range(12):
                            nc.tensor.matmul(
                                agg_ps,
                                lhsT=ag_sb[:, i, :],
                                rhs=x_sb[:, i * F + fc * FC:
                                         i * F + (fc + 1) * FC],
                                start=(i == 0),
                                stop=(i == 11),
                            )
                        hagg = hp.tile([SUB, FC], f32r, tag="hagg")
                        nc.scalar.copy(out=hagg, in_=agg_ps)
                        if dbg and st == 0:
                            nc.sync.dma_start(
                                out=bass.AP(dbg_hagg, fc * FC,
                                            [[F, SUB], [1, FC]]),
                                in_=hagg,
                            )
                        tr_ps = psT.tile([P, 3, SUB], f32r, tag="tr")
                        for j in range(3):
                            nc.tensor.transpose(
                                tr_ps[:, j, :],
                                hagg[:, j * P:(j + 1) * P],
                                id_sb,
                            )
                        nc.vector.tensor_copy(
                            out=haggT[:, fc * 3:(fc + 1) * 3,
                                      sl * SUB:(sl + 1) * SUB],
                            in_=tr_ps,
                        )
                # ---- w1 matmul + bias + relu ----
                mm1_ps = ps1.tile([P, 2, 512], f32, tag="mm1")
                for mh in range(2):
                    for kf in range(6):
                        nc.tensor.matmul(
                            mm1_ps[:, mh, :W],
                            lhsT=w1_sb[:, kf, mh * P:(mh + 1) * P],
                            rhs=haggT[:, kf, :W],
                            start=(kf == 0),
                            stop=(kf == 5),
                        )
                h2 = hp.tile([P, 2, 3 * SUB], f32r, tag="h2")
                for mh in range(2):
                    nc.scalar.activation(
                        out=h2[:, mh, :W],
                        in_=mm1_ps[:, mh, :W],
                        func=AF.Relu,
                        bias=b1_sb[:, mh:mh + 1],
                        scale=1.0,
                    )
                # ---- w2 matmul + bias + relu ----
                mm2_ps = ps2.tile([P, 2, 512], f32, tag="mm2")
                for mg in range(2):
                    for kh in range(2):
                        nc.tensor.matmul(
                            mm2_ps[:, mg, :W],
                            lhsT=w2_sb[:, kh, mg * P:(mg + 1) * P],
                            rhs=h2[:, kh, :W],
                            start=(kh == 0),
                            stop=(kh == 1),
                        )
                h3 = hp.tile([P, 2, 3 * SUB], f32, tag="h3")
                for mg in range(2):
                    nc.scalar.activation(
                        out=h3[:, mg, :W],
                        in_=mm2_ps[:, mg, :W],
                        func=AF.Relu,
                        bias=b2_sb[:, mg:mg + 1],
                        scale=1.0,
                    )
                if dbg and mi == 0:
                    nc.sync.dma_start(out=dbg_h3[:, :, :W], in_=h3[:, :, :W])
                if dbg and mi == 1:
                    nc.sync.dma_start(out=dbg_h3b[:, :, :W], in_=h3[:, :, :W])
                # ---- masked pooling (both segment slots) ----
                for s in range(2):
                    h3m = hp.tile([P, 2, 3 * SUB], f32, tag="h3m")
                    for mg in range(2):
                        nc.vector.tensor_mul(
                            out=h3m[:, mg, :W],
                            in0=h3[:, mg, :W],
                            in1=mk_sb[:, s, s0 * SUB:s0 * SUB + W],
                        )
                    nc.vector.reduce_sum(
                        out=acc_sb[:, s, :, mi],
                        in_=h3m[:, :, :W],
                        axis=AX.X,
                    )

            # ---- finale: reduce accumulators, classifier, write out ----
            if dbg:
                nc.sync.dma_start(out=dbg_mask[:, :, :], in_=mk_sb)
                nc.sync.dma_start(out=dbg_acc[:, :, :, :], in_=acc_sb)
            for s in range(2):
                pooled = fin.tile([P, 2], f32, tag="pooled")
                for kg in range(2):
                    nc.vector.reduce_sum(
                        out=pooled[:, kg:kg + 1],
                        in_=acc_sb[:, s, kg, :],
                        axis=AX.X,
                    )
                if dbg:
                    nc.sync.dma_start(
                        out=bass.AP(dbg_pool, s * 2, [[4, P], [1, 2]]),
                        in_=pooled,
                    )
                mm3_ps = ps3.tile([NL, 1], f32, tag="mm3")
                for kg in range(2):
                    nc.tensor.matmul(
                        mm3_ps,
                        lhsT=w3_sb[:, kg, :],
                        rhs=pooled[:, kg:kg + 1],
                        start=(kg == 0),
                        stop=(kg == 1),
                    )
                o_sb = fin.tile([NL, 1], f32, tag="osb")
                nc.scalar.add(out=o_sb, in_=mm3_ps, add=b3_sb)
                nc.sync.dma_start(
                    out=bass.AP(out_h, s * NL, [[1, NL]]),
                    in_=o_sb,
                )


def _prep_shared(aggr_w, w1, b1, w2, b2, w3, b3):
    aggw = np.zeros((12, SUB, SUB), dtype=np.float32)
    for i in range(12):
        for gt in range(G):
            for l in range(L):
                aggw[i, gt * L + l, i * G + gt] = aggr_w[l]
    w1t = np.ascontiguousarray(
        w1.T.reshape(6, P, H).transpose(1, 0, 2)).astype(np.float32)
    w2t = np.ascontiguousarray(
        w2.T.reshape(2, P, H).transpose(1, 0, 2)).astype(np.float32)
    w3t = np.ascontiguousarray(
        w3.T.reshape(2, P, NL).transpose(1, 0, 2)).astype(np.float32)
    b1s = np.ascontiguousarray(b1.reshape(2, P).T).astype(np.float32)
    b2s = np.ascontiguousarray(b2.reshape(2, P).T).astype(np.float32)
    b3s = b3.reshape(NL, 1).astype(np.float32)
    ident = np.eye(SUB, dtype=np.float32)
    return {
        "aggw": aggw, "w1t": w1t, "w2t": w2t, "b1s": b1s, "b2s": b2s,
        "w3t": w3t, "b3s": b3s, "ident": ident,
    }


def kernel(x, lengths, aggr_w, w1, b1, w2, b2, w3, b3):
    global LAST_RESULTS
    from concourse.bass_utils import run_bass_kernel_spmd

    x = np.asarray(x, dtype=np.float32)
    lens = np.asarray(lengths).astype(np.int64)
    aggr_w = np.asarray(aggr_w, dtype=np.float32)
    w1 = np.asarray(w1, dtype=np.float32)
    b1 = np.asarray(b1, dtype=np.float32)
    w2 = np.asarray(w2, dtype=np.float32)
    b2 = np.asarray(b2, dtype=np.float32)
    w3 = np.asarray(w3, dtype=np.float32)
    b3 = np.asarray(b3, dtype=np.float32)

    # pair longest with shortest to balance per-core work
    order = np.argsort(-lens, kind="stable")
    pairs = [(int(order[i]), int(order[B - 1 - i])) for i in range(NCORES)]
    psum_max = max(int(lens[a] + lens[b]) for a, b in pairs)
    tpad = max(SUB, ((psum_max + SUB - 1) // SUB) * SUB)
    nt10 = tpad // G

    if tpad not in _CACHE:
        _CACHE[tpad] = _build_bass(tpad)
    nc = _CACHE[tpad]

    shared = _prep_shared(aggr_w, w1, b1, w2, b2, w3, b3)
    in_maps = []
    for a, b in pairs:
        la, lb = int(lens[a]), int(lens[b])
        xt = np.zeros((L, tpad, F), dtype=np.float32)
        xt[:, :la] = x[a, :, :la]
        xt[:, la:la + lb] = x[b, :, :lb]
        # xq[gt, l, g, f] = xt[l, 10*g + gt, f]
        xq = np.ascontiguousarray(
            xt.reshape(L, nt10, G, F).transpose(2, 0, 1, 3))
        masks = np.zeros((2, tpad), dtype=np.float32)
        masks[0, :la] = 1.0 / la
        masks[1, la:la + lb] = 1.0 / lb
        in_maps.append({"x": xq, "masks": masks, **shared})

    res = run_bass_kernel_spmd(nc, in_maps, core_ids=list(range(NCORES)))
    LAST_RESULTS = res

    out = np.zeros((B, NL), dtype=np.float32)
    for c, (a, b) in enumerate(pairs):
        out[a] = res.results[c]["out"][0]
        out[b] = res.results[c]["out"][1]
    return out


# revision 21
# speedup vs baseline: 22100.9204x; 1.1901x over previous
"""Trainium2 Bass kernel for nn_Dense_1322849927863 (segment_reduce).

Reference computation:
  h   = einsum('bltf,l->btf', x, aggr_w)            # layer aggregation (L=12)
  h   = relu(h @ w1.T + b1)                         # [B,T,H=256]
  h   = relu(h @ w2.T + b2)                         # [B,T,256]
  pooled = (h * mask).sum(t) / lengths              # masked mean over t<len
  out = pooled @ w3.T + b3                          # [B,8]

Strategy (8 NeuronCores, data parallel over batch):
  - Host pairs the 16 batches (longest+shortest valid length) to balance
    per-core work and packs ONLY the valid t-rows of each pair into a dense
    buffer per core (masked rows never influence the output).  The packed
    buffer is laid out as xq[gt, l, g, f] with t = 10*g + gt so that one
    120-partition, 2-dim-AP DMA (36 KiB contiguous per partition) loads a
    full 120-t-row sub-tile as [partition=(gt,l), free=(g,f)].
  - Layer aggregation = 12 accumulating float32r matmuls per sub-tile with
    shifted block-diagonal stationary matrices -> hagg[t,f] in PSUM
    (float32r moves 1 column/cycle at N>=256; plain fp32 needs 4).
  - TensorE transposes flip hagg to [f,t]; two matmul chains apply w1/w2
    with fused bias+relu on ScalarE; masked pooling = DVE multiply +
    free-axis reduce with host-prepared (t<len)/len masks per segment slot.
    The 8-way classifier runs on-chip; host just reorders [2,8] per core.
"""

import numpy as np

B, L, T, F = 16, 12, 1024, 768
H, NL = 256, 8
NCORES = 8
P = 128
G = 10           # t-positions per aggregation group
SUB = 120        # t-rows per sub-tile (12 groups of 10), K = 120
FC = 384         # f columns per aggregation PSUM tile (2 chunks = 768)

_CACHE = {}
LAST_RESULTS = None  # BassKernelResults from the most recent run (for test.py)


def _macro_split(ns):
    """Group sub-tiles into macro tiles of >=2 where possible (N>=256 keeps
    float32r at full speed; a single short tail macro is negligible)."""
    macros = []
    s = 0
    while ns - s > 4:
        macros.append((s, 3))
        s += 3
    if ns - s == 4:
        macros.extend([(s, 2), (s + 2, 2)])
    elif ns - s > 0:
        macros.append((s, ns - s))
    return macros


def _build_bass(tpad, dbg=False, reps=0):
    import concourse.bass as bass
    import concourse.mybir as mybir
    import concourse.tile as tile
    from concourse import bacc

    f32 = mybir.dt.float32
    f32r = mybir.dt.float32r
    AF = mybir.ActivationFunctionType
    AX = mybir.AxisListType

    ns = tpad // SUB
    nt10 = tpad // G
    macros = _macro_split(ns)
    nmac = len(macros)

    nc = bacc.Bacc()
    x_h = nc.dram_tensor("x", [G, L, nt10, F], f32r, kind="ExternalInput")
    mk_h = nc.dram_tensor("masks", [2, tpad], f32, kind="ExternalInput")
    ag_h = nc.dram_tensor("aggw", [12, SUB, SUB], f32r, kind="ExternalInput")
    w1_h = nc.dram_tensor("w1t", [P, 6, H], f32r, kind="ExternalInput")
    w2_h = nc.dram_tensor("w2t", [P, 2, H], f32r, kind="ExternalInput")
    b1_h = nc.dram_tensor("b1s", [P, 2], f32, kind="ExternalInput")
    b2_h = nc.dram_tensor("b2s", [P, 2], f32, kind="ExternalInput")
    w3_h = nc.dram_tensor("w3t", [P, 2, NL], f32, kind="ExternalInput")
    b3_h = nc.dram_tensor("b3s", [NL, 1], f32, kind="ExternalInput")
    id_h = nc.dram_tensor("ident", [SUB, SUB], f32r, kind="ExternalInput")
    out_h = nc.dram_tensor("out", [2, NL], f32, kind="ExternalOutput")
    if dbg:
        dbg_hagg = nc.dram_tensor("dbg_hagg", [SUB, F], f32r, kind="ExternalOutput")
        dbg_h3 = nc.dram_tensor("dbg_h3", [P, 2, 3 * SUB], f32, kind="ExternalOutput")
        dbg_mask = nc.dram_tensor("dbg_mask", [P, 2, tpad], f32, kind="ExternalOutput")
        dbg_pool = nc.dram_tensor("dbg_pool", [P, 2, 2], f32, kind="ExternalOutput")
        nmac_ = len(_macro_split(tpad // SUB))
        dbg_acc = nc.dram_tensor("dbg_acc", [P, 2, 2, nmac_], f32, kind="ExternalOutput")
        dbg_h3b = nc.dram_tensor("dbg_h3b", [P, 2, 3 * SUB], f32, kind="ExternalOutput")

    with tile.TileContext(nc) as tc:
        with (
            tc.tile_pool(name="const", bufs=1) as const,
            tc.tile_pool(name="xp", bufs=3) as xp,
            tc.tile_pool(name="hp", bufs=2) as hp,
            tc.tile_pool(name="tp", bufs=2) as tp,
            tc.tile_pool(name="fin", bufs=2) as fin,
            tc.tile_pool(name="psA", bufs=2, space="PSUM") as psA,
            tc.tile_pool(name="psT", bufs=1, space="PSUM") as psT,
            tc.tile_pool(name="ps1", bufs=1, space="PSUM") as ps1,
            tc.tile_pool(name="ps2", bufs=1, space="PSUM") as ps2,
            tc.tile_pool(name="ps3", bufs=1, space="PSUM") as ps3,
        ):
            # ---- constants into SBUF ----
            ag_sb = const.tile([SUB, 12, SUB], f32r)
            nc.sync.dma_start(
                out=ag_sb,
                in_=bass.AP(ag_h, 0, [[SUB, SUB], [SUB * SUB, 12], [1, SUB]]),
            )
            w1_sb = const.tile([P, 6, H], f32r)
            nc.sync.dma_start(out=w1_sb, in_=w1_h[:, :, :])
            w2_sb = const.tile([P, 2, H], f32r)
            nc.sync.dma_start(out=w2_sb, in_=w2_h[:, :, :])
            b1_sb = const.tile([P, 2], f32)
            nc.sync.dma_start(out=b1_sb, in_=b1_h[:, :])
            b2_sb = const.tile([P, 2], f32)
            nc.sync.dma_start(out=b2_sb, in_=b2_h[:, :])
            w3_sb = const.tile([P, 2, NL], f32)
            nc.sync.dma_start(out=w3_sb, in_=w3_h[:, :, :])
            b3_sb = const.tile([NL, 1], f32)
            nc.sync.dma_start(out=b3_sb, in_=b3_h[:, :])
            id_sb = const.tile([SUB, SUB], f32r)
            nc.sync.dma_start(out=id_sb, in_=id_h[:, :])
            mk_sb = const.tile([P, 2, tpad], f32)
            for s in range(2):
                nc.gpsimd.dma_start(
                    out=mk_sb[:, s, :],
                    in_=bass.AP(mk_h, s * tpad, [[0, P], [1, tpad]]),
                )
            acc_sb = const.tile([P, 2, 2, nmac], f32)

            import contextlib
            rep_ctx = tc.For_i(0, reps, 1) if reps else contextlib.nullcontext()
            with rep_ctx:
                _emit_body(nc, tc, bass, mybir, tpad, macros, dbg,
                           locals())
    nc.compile()
    return nc


def _emit_body(nc, tc, bass, mybir, tpad, macros, dbg, env):
    f32 = mybir.dt.float32
    f32r = mybir.dt.float32r
    AF = mybir.ActivationFunctionType
    AX = mybir.AxisListType
    nt10 = tpad // G
    (const, xp, hp, tp, fin, psA, psT, ps1, ps2, ps3) = (
        env[k] for k in
        ("const", "xp", "hp", "tp", "fin", "psA", "psT", "ps1", "ps2", "ps3"))
    ag_sb, w1_sb, w2_sb, b1_sb, b2_sb, w3_sb, b3_sb, id_sb, mk_sb, acc_sb = (
        env[k] for k in ("ag_sb", "w1_sb", "w2_sb", "b1_sb", "b2_sb",
                         "w3_sb", "b3_sb", "id_sb", "mk_sb", "acc_sb"))
    x_h, out_h = env["x_h"], env["out_h"]
    if dbg:
        dbg_hagg, dbg_h3, dbg_mask, dbg_pool = (
            env[k] for k in ("dbg_hagg", "dbg_h3", "dbg_mask", "dbg_pool"))
        dbg_acc, dbg_h3b = env["dbg_acc"], env["dbg_h3b"]
    if True:
        if True:
            # ---- main loop over macro tiles ----
            for mi, (s0, msubs) in enumerate(macros):
                W = msubs * SUB
                haggT = tp.tile([P, 6, 3 * SUB], f32r, tag="haggT")
                for sl in range(msubs):
                    st = s0 + sl
                    x_sb = xp.tile([SUB, 12 * F], f32r, tag="x")
                    nc.sync.dma_start(
                        out=x_sb,
                        in_=bass.AP(
                            x_h, 12 * st * F, [[nt10 * F, SUB], [1, 12 * F]]
                        ),
                    )
                    for fc in range(2):
                        agg_ps = psA.tile([SUB, FC], f32, tag="agg")
                        for i in range(12):
                            nc.tensor.matmul(
                                agg_ps,
                                lhsT=ag_sb[:, i, :],
                                rhs=x_sb[:, i * F + fc * FC:
                                         i * F + (fc + 1) * FC],
                                start=(i == 0),
                                stop=(i == 11),
                            )
                        hagg = hp.tile([SUB, FC], f32r, tag="hagg")
                        nc.scalar.copy(out=hagg, in_=agg_ps)
                        if dbg and st == 0:
                            nc.sync.dma_start(
                                out=bass.AP(dbg_hagg, fc * FC,
                                            [[F, SUB], [1, FC]]),
                                in_=hagg,
                            )
                        tr_ps = psT.tile([P, 3, SUB], f32r, tag="tr")
                        for j in range(3):
                            nc.tensor.transpose(
                                tr_ps[:, j, :],
                                hagg[:, j * P:(j + 1) * P],
                                id_sb,
                            )
                        nc.vector.tensor_copy(
                            out=haggT[:, fc * 3:(fc + 1) * 3,
                                      sl * SUB:(sl + 1) * SUB],
                            in_=tr_ps,
                        )
                # ---- w1 matmul + bias + relu ----
                mm1_ps = ps1.tile([P, 2, 512], f32, tag="mm1")
                for mh in range(2):
                    for kf in range(6):
                        nc.tensor.matmul(
                            mm1_ps[:, mh, :W],
                            lhsT=w1_sb[:, kf, mh * P:(mh + 1) * P],
                            rhs=haggT[:, kf, :W],
                            start=(kf == 0),
                            stop=(kf == 5),
                        )
                h2 = hp.tile([P, 2, 3 * SUB], f32r, tag="h2")
                for mh in range(2):
                    nc.scalar.activation(
                        out=h2[:, mh, :W],
                        in_=mm1_ps[:, mh, :W],
                        func=AF.Relu,
                        bias=b1_sb[:, mh:mh + 1],
                        scale=1.0,
                    )
                # ---- w2 matmul + bias + relu ----
                mm2_ps = ps2.tile([P, 2, 512], f32, tag="mm2")
                for mg in range(2):
                    for kh in range(2):
                        nc.tensor.matmul(
                            mm2_ps[:, mg, :W],
                            lhsT=w2_sb[:, kh, mg * P:(mg + 1) * P],
                            rhs=h2[:, kh, :W],
                            start=(kh == 0),
                            stop=(kh == 1),
                        )
                h3 = hp.tile([P, 2, 3 * SUB], f32, tag="h3")
                for mg in range(2):
                    nc.scalar.activation(
                        out=h3[:, mg, :W],
                        in_=mm2_ps[:, mg, :W],
                        func=AF.Relu,
                        bias=b2_sb[:, mg:mg + 1],
                        scale=1.0,
                    )
                if dbg and mi == 0:
                    nc.sync.dma_start(out=dbg_h3[:, :, :W], in_=h3[:, :, :W])
                if dbg and mi == 1:
                    nc.sync.dma_start(out=dbg_h3b[:, :, :W], in_=h3[:, :, :W])
                # ---- masked pooling (both segment slots) ----
                for s in range(2):
                    h3m = hp.tile([P, 2, 3 * SUB], f32, tag="h3m")
                    for mg in range(2):
                        nc.vector.tensor_mul(
                            out=h3m[:, mg, :W],
                            in0=h3[:, mg, :W],
                            in1=mk_sb[:, s, s0 * SUB:s0 * SUB + W],
                        )
                    nc.vector.reduce_sum(
                        out=acc_sb[:, s, :, mi],
                        in_=h3m[:, :, :W],
                        axis=AX.X,
                    )

            # ---- finale: reduce accumulators, classifier, write out ----
            if dbg:
                nc.sync.dma_start(out=dbg_mask[:, :, :], in_=mk_sb)
                nc.sync.dma_start(out=dbg_acc[:, :, :, :], in_=acc_sb)
            for s in range(2):
                pooled = fin.tile([P, 2], f32, tag="pooled")
                for kg in range(2):
                    nc.vector.reduce_sum(
                        out=pooled[:, kg:kg + 1],
                        in_=acc_sb[:, s, kg, :],
                        axis=AX.X,
                    )
                if dbg:
                    nc.sync.dma_start(
                        out=bass.AP(dbg_pool, s * 2, [[4, P], [1, 2]]),
                        in_=pooled,
                    )
                mm3_ps = ps3.tile([NL, 1], f32, tag="mm3")
                for kg in range(2):
                    nc.tensor.matmul(
                        mm3_ps,
                        lhsT=w3_sb[:, kg, :],
                        rhs=pooled[:, kg:kg + 1],
                        start=(kg == 0),
                        stop=(kg == 1),
                    )
                o_sb = fin.tile([NL, 1], f32, tag="osb")
                nc.scalar.add(out=o_sb, in_=mm3_ps, add=b3_sb)
                nc.sync.dma_start(
                    out=bass.AP(out_h, s * NL, [[1, NL]]),
                    in_=o_sb,
                )


def _prep_shared(aggr_w, w1, b1, w2, b2, w3, b3):
    aggw = np.zeros((12, SUB, SUB), dtype=np.float32)
    for i in range(12):
        for gt in range(G):
            for l in range(L):
                aggw[i, gt * L + l, i * G + gt] = aggr_w[l]
    w1t = np.ascontiguousarray(
        w1.T.reshape(6, P, H).transpose(1, 0, 2)).astype(np.float32)
    w2t = np.ascontiguousarray(
        w2.T.reshape(2, P, H).transpose(1, 0, 2)).astype(np.float32)
    w3t = np.ascontiguousarray(
        w3.T.reshape(2, P, NL).transpose(1, 0, 2)).astype(np.float32)
    b1s = np.ascontiguousarray(b1.reshape(2, P).T).astype(np.float32)
    b2s = np.ascontiguousarray(b2.reshape(2, P).T).astype(np.float32)
    b3s = b3.reshape(NL, 1).astype(np.float32)
    ident = np.eye(SUB, dtype=np.float32)
    return {
        "aggw": aggw, "w1t": w1t, "w2t": w2t, "b1s": b1s, "b2s": b2s,
        "w3t": w3t, "b3s": b3s, "ident": ident,
    }


def kernel(x, lengths, aggr_w, w1, b1, w2, b2, w3, b3):
    global LAST_RESULTS
    from concourse.bass_utils import run_bass_kernel_spmd

    x = np.asarray(x, dtype=np.float32)
    lens = np.asarray(lengths).astype(np.int64)
    aggr_w = np.asarray(aggr_w, dtype=np.float32)
    w1 = np.asarray(w1, dtype=np.float32)
    b1 = np.asarray(b1, dtype=np.float32)
    w2 = np.asarray(w2, dtype=np.float32)
    b2 = np.asarray(b2, dtype=np.float32)
    w3 = np.asarray(w3, dtype=np.float32)
    b3 = np.asarray(b3, dtype=np.float32)

    # pair longest with shortest to balance per-core work
    order = np.argsort(-lens, kind="stable")
    pairs = [(int(order[i]), int(order[B - 1 - i])) for i in range(NCORES)]
    psum_max = max(int(lens[a] + lens[b]) for a, b in pairs)
    tpad = max(SUB, ((psum_max + SUB - 1) // SUB) * SUB)
    nt10 = tpad // G

    if tpad not in _CACHE:
        _CACHE[tpad] = _build_bass(tpad)
    nc = _CACHE[tpad]

    shared = _prep_shared(aggr_w, w1, b1, w2, b2, w3, b3)
    in_maps = []
    for a, b in pairs:
        la, lb = int(lens[a]), int(lens[b])
        xt = np.zeros((L, tpad, F), dtype=np.float32)
        xt[:, :la] = x[a, :, :la]
        xt[:, la:la + lb] = x[b, :, :lb]
        # xq[gt, l, g, f] = xt[l, 10*g + gt, f]
        xq = np.ascontiguousarray(
            xt.reshape(L, nt10, G, F).transpose(2, 0, 1, 3))
        masks = np.zeros((2, tpad), dtype=np.float32)
        masks[0, :la] = 1.0 / la
        masks[1, la:la + lb] = 1.0 / lb
        in_maps.append({"x": xq, "masks": masks, **shared})

    res = run_bass_kernel_spmd(nc, in_maps, core_ids=list(range(NCORES)))
    LAST_RESULTS = res

    out = np.zeros((B, NL), dtype=np.float32)
    for c, (a, b) in enumerate(pairs):
        out[a] = res.results[c]["out"][0]
        out[b] = res.results[c]["out"][1]
    return out


# revision 30
# speedup vs baseline: 29135.2074x; 1.3183x over previous
"""Trainium2 Bass kernel for nn_Dense_1322849927863 (segment_reduce).

Reference computation:
  h   = einsum('bltf,l->btf', x, aggr_w)            # layer aggregation (L=12)
  h   = relu(h @ w1.T + b1)                         # [B,T,H=256]
  h   = relu(h @ w2.T + b2)                         # [B,T,256]
  pooled = (h * mask).sum(t) / lengths              # masked mean over t<len
  out = pooled @ w3.T + b3                          # [B,8]

Strategy (8 NeuronCores, data parallel over batch):
  - Host pairs the 16 batches (longest+shortest valid length) to balance
    per-core work and packs ONLY the valid t-rows of each pair into a dense
    buffer per core (masked rows never influence the output).  The packed
    buffer is laid out as xq[gt, l, g, f] with t = 10*g + gt so that one
    120-partition, 2-dim-AP DMA (36 KiB contiguous per partition) loads a
    full 120-t-row sub-tile as [partition=(gt,l), free=(g,f)].
  - Layer aggregation = 12 accumulating float32r matmuls per sub-tile with
    shifted block-diagonal stationary matrices -> hagg[t,f] in PSUM
    (float32r moves 1 column/cycle at N>=256; plain fp32 needs 4).
  - TensorE transposes flip hagg to [f,t]; two matmul chains apply w1/w2
    with fused bias+relu on ScalarE; masked pooling = DVE multiply +
    free-axis reduce with host-prepared (t<len)/len masks per segment slot.
    The 8-way classifier runs on-chip; host just reorders [2,8] per core.
"""

import numpy as np

B, L, T, F = 16, 12, 1024, 768
H, NL = 256, 8
NCORES = 8
P = 128
G = 10           # t-positions per aggregation group
SUB = 120        # t-rows per sub-tile (12 groups of 10), K = 120
FC = 384         # f columns per aggregation PSUM tile (2 chunks = 768)

X_BF16 = True   # stream x (and the aggregation weights) as bf16: halves the
                # DMA roofline; aggregation still accumulates in fp32 PSUM.

_CACHE = {}
LAST_RESULTS = None  # BassKernelResults from the most recent run (for test.py)


def _macro_split(ns):
    """Group sub-tiles into macro tiles of >=2 where possible (N>=256 keeps
    float32r at full speed; a single short tail macro is negligible)."""
    macros = []
    s = 0
    while ns - s > 4:
        macros.append((s, 3))
        s += 3
    if ns - s == 4:
        macros.extend([(s, 2), (s + 2, 2)])
    elif ns - s > 0:
        macros.append((s, ns - s))
    return macros


def _build_bass(tpad, dbg=False, reps=0):
    import concourse.bass as bass
    import concourse.mybir as mybir
    import concourse.tile as tile
    from concourse import bacc

    f32 = mybir.dt.float32
    f32r = mybir.dt.float32r
    xdt = mybir.dt.bfloat16 if X_BF16 else f32r
    AF = mybir.ActivationFunctionType
    AX = mybir.AxisListType

    ns = tpad // SUB
    nt10 = tpad // G
    macros = _macro_split(ns)
    nmac = len(macros)

    nc = bacc.Bacc()
    x_h = nc.dram_tensor("x", [G, L, nt10, F], xdt, kind="ExternalInput")
    mk_h = nc.dram_tensor("masks", [2, tpad], f32, kind="ExternalInput")
    ag_h = nc.dram_tensor("aggw", [12, SUB, SUB], xdt, kind="ExternalInput")
    w1_h = nc.dram_tensor("w1t", [P, 6, H], f32r, kind="ExternalInput")
    w2_h = nc.dram_tensor("w2t", [P, 2, H], f32r, kind="ExternalInput")
    b1_h = nc.dram_tensor("b1s", [P, 2], f32, kind="ExternalInput")
    b2_h = nc.dram_tensor("b2s", [P, 2], f32, kind="ExternalInput")
    w3_h = nc.dram_tensor("w3t", [P, 2, NL], f32, kind="ExternalInput")
    b3_h = nc.dram_tensor("b3s", [NL, 1], f32, kind="ExternalInput")
    id_h = nc.dram_tensor("ident", [SUB, SUB], f32r, kind="ExternalInput")
    out_h = nc.dram_tensor("out", [2, NL], f32, kind="ExternalOutput")
    if dbg:
        dbg_hagg = nc.dram_tensor("dbg_hagg", [SUB, F], f32r, kind="ExternalOutput")
        dbg_h3 = nc.dram_tensor("dbg_h3", [P, 2, 3 * SUB], f32, kind="ExternalOutput")
        dbg_mask = nc.dram_tensor("dbg_mask", [P, 2, tpad], f32, kind="ExternalOutput")
        dbg_pool = nc.dram_tensor("dbg_pool", [P, 2, 2], f32, kind="ExternalOutput")
        nmac_ = len(_macro_split(tpad // SUB))
        dbg_acc = nc.dram_tensor("dbg_acc", [P, 2, 2, nmac_], f32, kind="ExternalOutput")
        dbg_h3b = nc.dram_tensor("dbg_h3b", [P, 2, 3 * SUB], f32, kind="ExternalOutput")

    with tile.TileContext(nc) as tc:
        with (
            tc.tile_pool(name="const", bufs=1) as const,
            tc.tile_pool(name="xp", bufs=4) as xp,
            tc.tile_pool(name="hp", bufs=2) as hp,
            tc.tile_pool(name="tp", bufs=2) as tp,
            tc.tile_pool(name="fin", bufs=2) as fin,
            tc.tile_pool(name="psA", bufs=2, space="PSUM") as psA,
            tc.tile_pool(name="psT", bufs=1, space="PSUM") as psT,
            tc.tile_pool(name="ps1", bufs=1, space="PSUM") as ps1,
            tc.tile_pool(name="ps2", bufs=1, space="PSUM") as ps2,
            tc.tile_pool(name="ps3", bufs=1, space="PSUM") as ps3,
        ):
            # ---- constants into SBUF ----
            ag_sb = const.tile([SUB, 12, SUB], xdt)
            nc.sync.dma_start(
                out=ag_sb,
                in_=bass.AP(ag_h, 0, [[SUB, SUB], [SUB * SUB, 12], [1, SUB]]),
            )
            w1_sb = const.tile([P, 6, H], f32r)
            nc.sync.dma_start(out=w1_sb, in_=w1_h[:, :, :])
            w2_sb = const.tile([P, 2, H], f32r)
            nc.sync.dma_start(out=w2_sb, in_=w2_h[:, :, :])
            b1_sb = const.tile([P, 2], f32)
            nc.sync.dma_start(out=b1_sb, in_=b1_h[:, :])
            b2_sb = const.tile([P, 2], f32)
            nc.sync.dma_start(out=b2_sb, in_=b2_h[:, :])
            w3_sb = const.tile([P, 2, NL], f32)
            nc.sync.dma_start(out=w3_sb, in_=w3_h[:, :, :])
            b3_sb = const.tile([NL, 1], f32)
            nc.sync.dma_start(out=b3_sb, in_=b3_h[:, :])
            id_sb = const.tile([SUB, SUB], f32r)
            nc.sync.dma_start(out=id_sb, in_=id_h[:, :])
            mk_sb = const.tile([P, 2, tpad], f32)
            for s in range(2):
                nc.gpsimd.dma_start(
                    out=mk_sb[:, s, :],
                    in_=bass.AP(mk_h, s * tpad, [[0, P], [1, tpad]]),
                )
            acc_sb = const.tile([P, 2, 2, nmac], f32)

            import contextlib
            rep_ctx = tc.For_i(0, reps, 1) if reps else contextlib.nullcontext()
            with rep_ctx:
                _emit_body(nc, tc, bass, mybir, tpad, macros, dbg,
                           locals())
    nc.compile()
    return nc


def _emit_body(nc, tc, bass, mybir, tpad, macros, dbg, env):
    f32 = mybir.dt.float32
    f32r = mybir.dt.float32r
    AF = mybir.ActivationFunctionType
    AX = mybir.AxisListType
    nt10 = tpad // G
    (const, xp, hp, tp, fin, psA, psT, ps1, ps2, ps3) = (
        env[k] for k in
        ("const", "xp", "hp", "tp", "fin", "psA", "psT", "ps1", "ps2", "ps3"))
    ag_sb, w1_sb, w2_sb, b1_sb, b2_sb, w3_sb, b3_sb, id_sb, mk_sb, acc_sb = (
        env[k] for k in ("ag_sb", "w1_sb", "w2_sb", "b1_sb", "b2_sb",
                         "w3_sb", "b3_sb", "id_sb", "mk_sb", "acc_sb"))
    x_h, out_h = env["x_h"], env["out_h"]
    if dbg:
        dbg_hagg, dbg_h3, dbg_mask, dbg_pool = (
            env[k] for k in ("dbg_hagg", "dbg_h3", "dbg_mask", "dbg_pool"))
        dbg_acc, dbg_h3b = env["dbg_acc"], env["dbg_h3b"]
    if True:
        if True:
            # ---- main loop over macro tiles ----
            for mi, (s0, msubs) in enumerate(macros):
                W = msubs * SUB
                haggT = tp.tile([P, 6, 3 * SUB], f32r, tag="haggT")
                for sl in range(msubs):
                    st = s0 + sl
                    x_sb = xp.tile([SUB, 12 * F],
                                   mybir.dt.bfloat16 if X_BF16 else f32r,
                                   tag="x")
                    nc.sync.dma_start(
                        out=x_sb,
                        in_=bass.AP(
                            x_h, 12 * st * F, [[nt10 * F, SUB], [1, 12 * F]]
                        ),
                    )
                    for fc in range(2):
                        agg_ps = psA.tile([SUB, FC], f32, tag="agg")
                        for i in range(12):
                            nc.tensor.matmul(
                                agg_ps,
                                lhsT=ag_sb[:, i, :],
                                rhs=x_sb[:, i * F + fc * FC:
                                         i * F + (fc + 1) * FC],
                                start=(i == 0),
                                stop=(i == 11),
                            )
                        hagg = hp.tile([SUB, FC], f32r, tag="hagg")
                        nc.scalar.copy(out=hagg, in_=agg_ps)
                        if dbg and st == 0:
                            nc.sync.dma_start(
                                out=bass.AP(dbg_hagg, fc * FC,
                                            [[F, SUB], [1, FC]]),
                                in_=hagg,
                            )
                        tr_ps = psT.tile([P, 3, SUB], f32r, tag="tr")
                        for j in range(3):
                            nc.tensor.transpose(
                                tr_ps[:, j, :],
                                hagg[:, j * P:(j + 1) * P],
                                id_sb,
                            )
                        nc.vector.tensor_copy(
                            out=haggT[:, fc * 3:(fc + 1) * 3,
                                      sl * SUB:(sl + 1) * SUB],
                            in_=tr_ps,
                        )
                # ---- w1 matmul + bias + relu ----
                mm1_ps = ps1.tile([P, 2, 512], f32, tag="mm1")
                for mh in range(2):
                    for kf in range(6):
                        nc.tensor.matmul(
                            mm1_ps[:, mh, :W],
                            lhsT=w1_sb[:, kf, mh * P:(mh + 1) * P],
                            rhs=haggT[:, kf, :W],
                            start=(kf == 0),
                            stop=(kf == 5),
                        )
                h2 = hp.tile([P, 2, 3 * SUB], f32r, tag="h2")
                for mh in range(2):
                    nc.scalar.activation(
                        out=h2[:, mh, :W],
                        in_=mm1_ps[:, mh, :W],
                        func=AF.Relu,
                        bias=b1_sb[:, mh:mh + 1],
                        scale=1.0,
                    )
                # ---- w2 matmul + bias + relu ----
                mm2_ps = ps2.tile([P, 2, 512], f32, tag="mm2")
                for mg in range(2):
                    for kh in range(2):
                        nc.tensor.matmul(
                            mm2_ps[:, mg, :W],
                            lhsT=w2_sb[:, kh, mg * P:(mg + 1) * P],
                            rhs=h2[:, kh, :W],
                            start=(kh == 0),
                            stop=(kh == 1),
                        )
                h3 = hp.tile([P, 2, 3 * SUB], f32, tag="h3")
                for mg in range(2):
                    nc.scalar.activation(
                        out=h3[:, mg, :W],
                        in_=mm2_ps[:, mg, :W],
                        func=AF.Relu,
                        bias=b2_sb[:, mg:mg + 1],
                        scale=1.0,
                    )
                if dbg and mi == 0:
                    nc.sync.dma_start(out=dbg_h3[:, :, :W], in_=h3[:, :, :W])
                if dbg and mi == 1:
                    nc.sync.dma_start(out=dbg_h3b[:, :, :W], in_=h3[:, :, :W])
                # ---- masked pooling (both segment slots) ----
                for s in range(2):
                    h3m = hp.tile([P, 2, 3 * SUB], f32, tag="h3m")
                    for mg in range(2):
                        nc.vector.tensor_mul(
                            out=h3m[:, mg, :W],
                            in0=h3[:, mg, :W],
                            in1=mk_sb[:, s, s0 * SUB:s0 * SUB + W],
                        )
                    nc.vector.reduce_sum(
                        out=acc_sb[:, s, :, mi],
                        in_=h3m[:, :, :W],
                        axis=AX.X,
                    )

            # ---- finale: reduce accumulators, classifier, write out ----
            if dbg:
                nc.sync.dma_start(out=dbg_mask[:, :, :], in_=mk_sb)
                nc.sync.dma_start(out=dbg_acc[:, :, :, :], in_=acc_sb)
            for s in range(2):
                pooled = fin.tile([P, 2], f32, tag="pooled")
                for kg in range(2):
                    nc.vector.reduce_sum(
                        out=pooled[:, kg:kg + 1],
                        in_=acc_sb[:, s, kg, :],
                        axis=AX.X,
                    )
                if dbg:
                    nc.sync.dma_start(
                        out=bass.AP(dbg_pool, s * 2, [[4, P], [1, 2]]),
                        in_=pooled,
                    )
                mm3_ps = ps3.tile([NL, 1], f32, tag="mm3")
                for kg in range(2):
                    nc.tensor.matmul(
                        mm3_ps,
                        lhsT=w3_sb[:, kg, :],
                        rhs=pooled[:, kg:kg + 1],
                        start=(kg == 0),
                        stop=(kg == 1),
                    )
                o_sb = fin.tile([NL, 1], f32, tag="osb")
                nc.scalar.add(out=o_sb, in_=mm3_ps, add=b3_sb)
                nc.sync.dma_start(
                    out=bass.AP(out_h, s * NL, [[1, NL]]),
                    in_=o_sb,
                )


def _xdt_np():
    if X_BF16:
        import ml_dtypes
        return ml_dtypes.bfloat16
    return np.float32


def _prep_shared(aggr_w, w1, b1, w2, b2, w3, b3):
    aggw = np.zeros((12, SUB, SUB), dtype=np.float32)
    for i in range(12):
        for gt in range(G):
            for l in range(L):
                aggw[i, gt * L + l, i * G + gt] = aggr_w[l]
    w1t = np.ascontiguousarray(
        w1.T.reshape(6, P, H).transpose(1, 0, 2)).astype(np.float32)
    w2t = np.ascontiguousarray(
        w2.T.reshape(2, P, H).transpose(1, 0, 2)).astype(np.float32)
    w3t = np.ascontiguousarray(
        w3.T.reshape(2, P, NL).transpose(1, 0, 2)).astype(np.float32)
    b1s = np.ascontiguousarray(b1.reshape(2, P).T).astype(np.float32)
    b2s = np.ascontiguousarray(b2.reshape(2, P).T).astype(np.float32)
    b3s = b3.reshape(NL, 1).astype(np.float32)
    ident = np.eye(SUB, dtype=np.float32)
    return {
        "aggw": aggw.astype(_xdt_np()), "w1t": w1t, "w2t": w2t,
        "b1s": b1s, "b2s": b2s, "w3t": w3t, "b3s": b3s, "ident": ident,
    }


def kernel(x, lengths, aggr_w, w1, b1, w2, b2, w3, b3):
    global LAST_RESULTS
    from concourse.bass_utils import run_bass_kernel_spmd

    x = np.asarray(x, dtype=np.float32)
    lens = np.asarray(lengths).astype(np.int64)
    aggr_w = np.asarray(aggr_w, dtype=np.float32)
    w1 = np.asarray(w1, dtype=np.float32)
    b1 = np.asarray(b1, dtype=np.float32)
    w2 = np.asarray(w2, dtype=np.float32)
    b2 = np.asarray(b2, dtype=np.float32)
    w3 = np.asarray(w3, dtype=np.float32)
    b3 = np.asarray(b3, dtype=np.float32)

    # pair longest with shortest to balance per-core work
    order = np.argsort(-lens, kind="stable")
    pairs = [(int(order[i]), int(order[B - 1 - i])) for i in range(NCORES)]
    psum_max = max(int(lens[a] + lens[b]) for a, b in pairs)
    tpad = max(SUB, ((psum_max + SUB - 1) // SUB) * SUB)
    nt10 = tpad // G

    if tpad not in _CACHE:
        _CACHE[tpad] = _build_bass(tpad)
    nc = _CACHE[tpad]

    shared = _prep_shared(aggr_w, w1, b1, w2, b2, w3, b3)
    in_maps = []
    for a, b in pairs:
        la, lb = int(lens[a]), int(lens[b])
        xt = np.zeros((L, tpad, F), dtype=np.float32)
        xt[:, :la] = x[a, :, :la]
        xt[:, la:la + lb] = x[b, :, :lb]
        # xq[gt, l, g, f] = xt[l, 10*g + gt, f]
        xq = np.ascontiguousarray(
            xt.reshape(L, nt10, G, F).transpose(2, 0, 1, 3)).astype(_xdt_np())
        masks = np.zeros((2, tpad), dtype=np.float32)
        masks[0, :la] = 1.0 / la
        masks[1, la:la + lb] = 1.0 / lb
        in_maps.append({"x": xq, "masks": masks, **shared})

    res = run_bass_kernel_spmd(nc, in_maps, core_ids=list(range(NCORES)))
    LAST_RESULTS = res

    out = np.zeros((B, NL), dtype=np.float32)
    for c, (a, b) in enumerate(pairs):
        out[a] = res.results[c]["out"][0]
        out[b] = res.results[c]["out"][1]
    return out


# revision 43
# speedup vs baseline: 39769.3013x; 1.3650x over previous
"""Trainium2 Bass kernel for nn_Dense_1322849927863 (segment_reduce).

Reference computation:
  h   = einsum('bltf,l->btf', x, aggr_w)            # layer aggregation (L=12)
  h   = relu(h @ w1.T + b1)                         # [B,T,H=256]
  h   = relu(h @ w2.T + b2)                         # [B,T,256]
  pooled = (h * mask).sum(t) / lengths              # masked mean over t<len
  out = pooled @ w3.T + b3                          # [B,8]

Strategy (8 NeuronCores, data parallel over batch):
  - Host pairs the 16 batches (longest+shortest valid length) to balance
    per-core work and packs ONLY the valid t-rows of each pair into a dense
    buffer per core (masked rows never influence the output).  The packed
    buffer is laid out as xq[gt, l, g, f] with t = 10*g + gt so that one
    120-partition, 2-dim-AP DMA (36 KiB contiguous per partition) loads a
    full 120-t-row sub-tile as [partition=(gt,l), free=(g,f)].
  - Layer aggregation = 12 accumulating float32r matmuls per sub-tile with
    shifted block-diagonal stationary matrices -> hagg[t,f] in PSUM
    (float32r moves 1 column/cycle at N>=256; plain fp32 needs 4).
  - TensorE transposes flip hagg to [f,t]; two matmul chains apply w1/w2
    with fused bias+relu on ScalarE; masked pooling = DVE multiply +
    free-axis reduce with host-prepared (t<len)/len masks per segment slot.
    The 8-way classifier runs on-chip; host just reorders [2,8] per core.
"""

import numpy as np

B, L, T, F = 16, 12, 1024, 768
H, NL = 256, 8
NCORES = 8
P = 128
G = 10           # t-positions per aggregation group
SUB = 120        # t-rows per sub-tile (12 groups of 10), K = 120
FC = 384         # f columns per aggregation PSUM tile (2 chunks = 768)

X_BF16 = True   # stream x (and the aggregation weights) as bf16: halves the
                # DMA roofline; aggregation still accumulates in fp32 PSUM.

_CACHE = {}
LAST_RESULTS = None  # BassKernelResults from the most recent run (for test.py)


def _macro_split(ns):
    """Group sub-tiles into macro tiles of >=2 where possible (N>=256 keeps
    float32r at full speed; a single short tail macro is negligible)."""
    macros = []
    s = 0
    while ns - s > 4:
        macros.append((s, 3))
        s += 3
    if ns - s == 4:
        macros.extend([(s, 2), (s + 2, 2)])
    elif ns - s > 0:
        macros.append((s, ns - s))
    return macros


def _build_bass(tpad, dbg=False, reps=0):
    import concourse.bass as bass
    import concourse.mybir as mybir
    import concourse.tile as tile
    from concourse import bacc

    f32 = mybir.dt.float32
    f32r = mybir.dt.float32r
    bf16 = mybir.dt.bfloat16
    xdt = bf16 if X_BF16 else f32r
    AF = mybir.ActivationFunctionType
    AX = mybir.AxisListType

    ns = tpad // SUB
    nt10 = tpad // G
    macros = _macro_split(ns)
    nmac = len(macros)

    nc = bacc.Bacc()
    x_h = nc.dram_tensor("x", [G, L, nt10, F], xdt, kind="ExternalInput")
    mk_h = nc.dram_tensor("masks", [2, tpad], f32, kind="ExternalInput")
    ag_h = nc.dram_tensor("aggw", [12, SUB, SUB], xdt, kind="ExternalInput")
    w1_h = nc.dram_tensor("w1t", [P, 6, H], bf16, kind="ExternalInput")
    w2_h = nc.dram_tensor("w2t", [P, 2, H], bf16, kind="ExternalInput")
    b1_h = nc.dram_tensor("b1s", [P, 2], f32, kind="ExternalInput")
    b2_h = nc.dram_tensor("b2s", [P, 2], f32, kind="ExternalInput")
    w3_h = nc.dram_tensor("w3t", [P, 2, NL], f32, kind="ExternalInput")
    b3_h = nc.dram_tensor("b3s", [NL, 1], f32, kind="ExternalInput")
    id_h = nc.dram_tensor("ident", [SUB, SUB], bf16, kind="ExternalInput")
    out_h = nc.dram_tensor("out", [2, NL], f32, kind="ExternalOutput")
    if dbg:
        dbg_hagg = nc.dram_tensor("dbg_hagg", [SUB, F], mybir.dt.bfloat16,
                                  kind="ExternalOutput")
        dbg_h3 = nc.dram_tensor("dbg_h3", [P, 2, 3 * SUB], f32, kind="ExternalOutput")
        dbg_mask = nc.dram_tensor("dbg_mask", [P, 2, tpad], f32, kind="ExternalOutput")
        dbg_pool = nc.dram_tensor("dbg_pool", [P, 2, 2], f32, kind="ExternalOutput")
        nmac_ = len(_macro_split(tpad // SUB))
        dbg_acc = nc.dram_tensor("dbg_acc", [P, 2, 2, nmac_], f32, kind="ExternalOutput")
        dbg_h3b = nc.dram_tensor("dbg_h3b", [P, 2, 3 * SUB], f32, kind="ExternalOutput")

    with tile.TileContext(nc) as tc:
        with (
            tc.tile_pool(name="const", bufs=1) as const,
            tc.tile_pool(name="xp", bufs=4) as xp,
            tc.tile_pool(name="hp", bufs=2) as hp,
            tc.tile_pool(name="tp", bufs=2) as tp,
            tc.tile_pool(name="fin", bufs=2) as fin,
            tc.tile_pool(name="psA", bufs=2, space="PSUM") as psA,
            tc.tile_pool(name="psT", bufs=1, space="PSUM") as psT,
            tc.tile_pool(name="ps1", bufs=1, space="PSUM") as ps1,
            tc.tile_pool(name="ps2", bufs=1, space="PSUM") as ps2,
            tc.tile_pool(name="ps3", bufs=1, space="PSUM") as ps3,
        ):
            # ---- constants into SBUF ----
            ag_sb = const.tile([SUB, 12, SUB], xdt)
            nc.sync.dma_start(
                out=ag_sb,
                in_=bass.AP(ag_h, 0, [[SUB, SUB], [SUB * SUB, 12], [1, SUB]]),
            )
            w1_sb = const.tile([P, 6, H], bf16)
            nc.sync.dma_start(out=w1_sb, in_=w1_h[:, :, :])
            w2_sb = const.tile([P, 2, H], bf16)
            nc.sync.dma_start(out=w2_sb, in_=w2_h[:, :, :])
            b1_sb = const.tile([P, 2], f32)
            nc.sync.dma_start(out=b1_sb, in_=b1_h[:, :])
            b2_sb = const.tile([P, 2], f32)
            nc.sync.dma_start(out=b2_sb, in_=b2_h[:, :])
            w3_sb = const.tile([P, 2, NL], f32)
            nc.sync.dma_start(out=w3_sb, in_=w3_h[:, :, :])
            b3_sb = const.tile([NL, 1], f32)
            nc.sync.dma_start(out=b3_sb, in_=b3_h[:, :])
            id_sb = const.tile([SUB, SUB], bf16)
            nc.sync.dma_start(out=id_sb, in_=id_h[:, :])
            mk_sb = const.tile([P, 2, tpad], f32)
            for s in range(2):
                nc.gpsimd.dma_start(
                    out=mk_sb[:, s, :],
                    in_=bass.AP(mk_h, s * tpad, [[0, P], [1, tpad]]),
                )
            acc_sb = const.tile([P, 2, 2, nmac], f32)

            import contextlib
            rep_ctx = tc.For_i(0, reps, 1) if reps else contextlib.nullcontext()
            with rep_ctx:
                _emit_body(nc, tc, bass, mybir, tpad, macros, dbg,
                           locals())
    nc.compile()
    return nc


def _emit_body(nc, tc, bass, mybir, tpad, macros, dbg, env):
    f32 = mybir.dt.float32
    f32r = mybir.dt.float32r
    bf16 = mybir.dt.bfloat16
    AF = mybir.ActivationFunctionType
    AX = mybir.AxisListType
    nt10 = tpad // G
    (const, xp, hp, tp, fin, psA, psT, ps1, ps2, ps3) = (
        env[k] for k in
        ("const", "xp", "hp", "tp", "fin", "psA", "psT", "ps1", "ps2", "ps3"))
    ag_sb, w1_sb, w2_sb, b1_sb, b2_sb, w3_sb, b3_sb, id_sb, mk_sb, acc_sb = (
        env[k] for k in ("ag_sb", "w1_sb", "w2_sb", "b1_sb", "b2_sb",
                         "w3_sb", "b3_sb", "id_sb", "mk_sb", "acc_sb"))
    x_h, out_h = env["x_h"], env["out_h"]
    if dbg:
        dbg_hagg, dbg_h3, dbg_mask, dbg_pool = (
            env[k] for k in ("dbg_hagg", "dbg_h3", "dbg_mask", "dbg_pool"))
        dbg_acc, dbg_h3b = env["dbg_acc"], env["dbg_h3b"]
    if True:
        if True:
            # ---- main loop over macro tiles ----
            for mi, (s0, msubs) in enumerate(macros):
                W = msubs * SUB
                haggT = tp.tile([P, 6, 3 * SUB], bf16, tag="haggT")
                for sl in range(msubs):
                    st = s0 + sl
                    x_sb = xp.tile([SUB, 12 * F],
                                   mybir.dt.bfloat16 if X_BF16 else f32r,
                                   tag="x")
                    nc.sync.dma_start(
                        out=x_sb,
                        in_=bass.AP(
                            x_h, 12 * st * F, [[nt10 * F, SUB], [1, 12 * F]]
                        ),
                    )
                    for fc in range(2):
                        agg_ps = psA.tile([SUB, FC], f32, tag="agg")
                        for i in range(12):
                            nc.tensor.matmul(
                                agg_ps,
                                lhsT=ag_sb[:, i, :],
                                rhs=x_sb[:, i * F + fc * FC:
                                         i * F + (fc + 1) * FC],
                                start=(i == 0),
                                stop=(i == 11),
                            )
                        hagg = hp.tile([SUB, FC], bf16, tag="hagg")
                        nc.scalar.copy(out=hagg, in_=agg_ps)
                        if dbg and st == 0:
                            nc.sync.dma_start(
                                out=bass.AP(dbg_hagg, fc * FC,
                                            [[F, SUB], [1, FC]]),
                                in_=hagg,
                            )
                        tr_ps = psT.tile([P, 3, SUB], bf16, tag="tr")
                        for j in range(3):
                            nc.tensor.transpose(
                                tr_ps[:, j, :],
                                hagg[:, j * P:(j + 1) * P],
                                id_sb,
                            )
                        nc.vector.tensor_copy(
                            out=haggT[:, fc * 3:(fc + 1) * 3,
                                      sl * SUB:(sl + 1) * SUB],
                            in_=tr_ps,
                        )
                # ---- w1 matmul + bias + relu ----
                mm1_ps = ps1.tile([P, 2, 512], f32, tag="mm1")
                for mh in range(2):
                    for kf in range(6):
                        nc.tensor.matmul(
                            mm1_ps[:, mh, :W],
                            lhsT=w1_sb[:, kf, mh * P:(mh + 1) * P],
                            rhs=haggT[:, kf, :W],
                            start=(kf == 0),
                            stop=(kf == 5),
                        )
                h2 = hp.tile([P, 2, 3 * SUB], bf16, tag="h2")
                for mh in range(2):
                    nc.scalar.activation(
                        out=h2[:, mh, :W],
                        in_=mm1_ps[:, mh, :W],
                        func=AF.Relu,
                        bias=b1_sb[:, mh:mh + 1],
                        scale=1.0,
                    )
                # ---- w2 matmul + bias + relu ----
                mm2_ps = ps2.tile([P, 2, 512], f32, tag="mm2")
                for mg in range(2):
                    for kh in range(2):
                        nc.tensor.matmul(
                            mm2_ps[:, mg, :W],
                            lhsT=w2_sb[:, kh, mg * P:(mg + 1) * P],
                            rhs=h2[:, kh, :W],
                            start=(kh == 0),
                            stop=(kh == 1),
                        )
                h3 = hp.tile([P, 2, 3 * SUB], f32, tag="h3")
                for mg in range(2):
                    nc.scalar.activation(
                        out=h3[:, mg, :W],
                        in_=mm2_ps[:, mg, :W],
                        func=AF.Relu,
                        bias=b2_sb[:, mg:mg + 1],
                        scale=1.0,
                    )
                if dbg and mi == 0:
                    nc.sync.dma_start(out=dbg_h3[:, :, :W], in_=h3[:, :, :W])
                if dbg and mi == 1:
                    nc.sync.dma_start(out=dbg_h3b[:, :, :W], in_=h3[:, :, :W])
                # ---- masked pooling (both segment slots) ----
                for s in range(2):
                    h3m = hp.tile([P, 2, 3 * SUB], f32, tag="h3m")
                    for mg in range(2):
                        nc.vector.tensor_mul(
                            out=h3m[:, mg, :W],
                            in0=h3[:, mg, :W],
                            in1=mk_sb[:, s, s0 * SUB:s0 * SUB + W],
                        )
                    nc.vector.reduce_sum(
                        out=acc_sb[:, s, :, mi],
                        in_=h3m[:, :, :W],
                        axis=AX.X,
                    )

            # ---- finale: reduce accumulators, classifier, write out ----
            if dbg:
                nc.sync.dma_start(out=dbg_mask[:, :, :], in_=mk_sb)
                nc.sync.dma_start(out=dbg_acc[:, :, :, :], in_=acc_sb)
            for s in range(2):
                pooled = fin.tile([P, 2], f32, tag="pooled")
                for kg in range(2):
                    nc.vector.reduce_sum(
                        out=pooled[:, kg:kg + 1],
                        in_=acc_sb[:, s, kg, :],
                        axis=AX.X,
                    )
                if dbg:
                    nc.sync.dma_start(
                        out=bass.AP(dbg_pool, s * 2, [[4, P], [1, 2]]),
                        in_=pooled,
                    )
                mm3_ps = ps3.tile([NL, 1], f32, tag="mm3")
                for kg in range(2):
                    nc.tensor.matmul(
                        mm3_ps,
                        lhsT=w3_sb[:, kg, :],
                        rhs=pooled[:, kg:kg + 1],
                        start=(kg == 0),
                        stop=(kg == 1),
                    )
                o_sb = fin.tile([NL, 1], f32, tag="osb")
                nc.scalar.add(out=o_sb, in_=mm3_ps, add=b3_sb)
                nc.sync.dma_start(
                    out=bass.AP(out_h, s * NL, [[1, NL]]),
                    in_=o_sb,
                )


def _xdt_np():
    if X_BF16:
        import ml_dtypes
        return ml_dtypes.bfloat16
    return np.float32


def _prep_shared(aggr_w, w1, b1, w2, b2, w3, b3):
    aggw = np.zeros((12, SUB, SUB), dtype=np.float32)
    for i in range(12):
        for gt in range(G):
            for l in range(L):
                aggw[i, gt * L + l, i * G + gt] = aggr_w[l]
    import ml_dtypes
    w1t = np.ascontiguousarray(
        w1.T.reshape(6, P, H).transpose(1, 0, 2)).astype(ml_dtypes.bfloat16)
    w2t = np.ascontiguousarray(
        w2.T.reshape(2, P, H).transpose(1, 0, 2)).astype(ml_dtypes.bfloat16)
    w3t = np.ascontiguousarray(
        w3.T.reshape(2, P, NL).transpose(1, 0, 2)).astype(np.float32)
    b1s = np.ascontiguousarray(b1.reshape(2, P).T).astype(np.float32)
    b2s = np.ascontiguousarray(b2.reshape(2, P).T).astype(np.float32)
    b3s = b3.reshape(NL, 1).astype(np.float32)
    ident = np.eye(SUB).astype(ml_dtypes.bfloat16)
    return {
        "aggw": aggw.astype(_xdt_np()), "w1t": w1t, "w2t": w2t,
        "b1s": b1s, "b2s": b2s, "w3t": w3t, "b3s": b3s, "ident": ident,
    }


def kernel(x, lengths, aggr_w, w1, b1, w2, b2, w3, b3):
    global LAST_RESULTS
    from concourse.bass_utils import run_bass_kernel_spmd

    x = np.asarray(x, dtype=np.float32)
    lens = np.asarray(lengths).astype(np.int64)
    aggr_w = np.asarray(aggr_w, dtype=np.float32)
    w1 = np.asarray(w1, dtype=np.float32)
    b1 = np.asarray(b1, dtype=np.float32)
    w2 = np.asarray(w2, dtype=np.float32)
    b2 = np.asarray(b2, dtype=np.float32)
    w3 = np.asarray(w3, dtype=np.float32)
    b3 = np.asarray(b3, dtype=np.float32)

    # pair longest with shortest to balance per-core work
    order = np.argsort(-lens, kind="stable")
    pairs = [(int(order[i]), int(order[B - 1 - i])) for i in range(NCORES)]
    psum_max = max(int(lens[a] + lens[b]) for a, b in pairs)
    tpad = max(SUB, ((psum_max + SUB - 1) // SUB) * SUB)
    nt10 = tpad // G

    if tpad not in _CACHE:
        _CACHE[tpad] = _build_bass(tpad)
    nc = _CACHE[tpad]

    shared = _prep_shared(aggr_w, w1, b1, w2, b2, w3, b3)
    in_maps = []
    for a, b in pairs:
        la, lb = int(lens[a]), int(lens[b])
        xt = np.zeros((L, tpad, F), dtype=np.float32)
        xt[:, :la] = x[a, :, :la]
        xt[:, la:la + lb] = x[b, :, :lb]
        # xq[gt, l, g, f] = xt[l, 10*g + gt, f]
        xq = np.ascontiguousarray(
            xt.reshape(L, nt10, G, F).transpose(2, 0, 1, 3)).astype(_xdt_np())
        masks = np.zeros((2, tpad), dtype=np.float32)
        masks[0, :la] = 1.0 / la
        masks[1, la:la + lb] = 1.0 / lb
        in_maps.append({"x": xq, "masks": masks, **shared})

    res = run_bass_kernel_spmd(nc, in_maps, core_ids=list(range(NCORES)))
    LAST_RESULTS = res

    out = np.zeros((B, NL), dtype=np.float32)
    for c, (a, b) in enumerate(pairs):
        out[a] = res.results[c]["out"][0]
        out[b] = res.results[c]["out"][1]
    return out


# revision 45
# speedup vs baseline: 49560.0440x; 1.2462x over previous
"""Trainium2 Bass kernel for nn_Dense_1322849927863 (segment_reduce).

Reference computation:
  h   = einsum('bltf,l->btf', x, aggr_w)            # layer aggregation (L=12)
  h   = relu(h @ w1.T + b1)                         # [B,T,H=256]
  h   = relu(h @ w2.T + b2)                         # [B,T,256]
  pooled = (h * mask).sum(t) / lengths              # masked mean over t<len
  out = pooled @ w3.T + b3                          # [B,8]

Strategy (8 NeuronCores, data parallel over batch):
  - Host pairs the 16 batches (longest+shortest valid length) to balance
    per-core work and packs ONLY the valid t-rows of each pair into a dense
    buffer per core (masked rows never influence the output).  The packed
    buffer is laid out as xq[gt, l, g, f] with t = 10*g + gt so that one
    120-partition, 2-dim-AP DMA (36 KiB contiguous per partition) loads a
    full 120-t-row sub-tile as [partition=(gt,l), free=(g,f)].
  - Layer aggregation = 12 accumulating float32r matmuls per sub-tile with
    shifted block-diagonal stationary matrices -> hagg[t,f] in PSUM
    (float32r moves 1 column/cycle at N>=256; plain fp32 needs 4).
  - TensorE transposes flip hagg to [f,t]; two matmul chains apply w1/w2
    with fused bias+relu on ScalarE; masked pooling = DVE multiply +
    free-axis reduce with host-prepared (t<len)/len masks per segment slot.
    The 8-way classifier runs on-chip; host just reorders [2,8] per core.
"""

import numpy as np

B, L, T, F = 16, 12, 1024, 768
H, NL = 256, 8
NCORES = 8
P = 128
G = 10           # t-positions per aggregation group
SUB = 120        # t-rows per sub-tile (12 groups of 10), K = 120
FC = 384         # f columns per aggregation PSUM tile (2 chunks = 768)

X_BF16 = True   # stream x (and the aggregation weights) as bf16: halves the
                # DMA roofline; aggregation still accumulates in fp32 PSUM.

_CACHE = {}
LAST_RESULTS = None  # BassKernelResults from the most recent run (for test.py)


def _macro_split(ns):
    """Group sub-tiles into macro tiles of >=2 where possible (N>=256 keeps
    float32r at full speed; a single short tail macro is negligible)."""
    macros = []
    s = 0
    while ns - s > 4:
        macros.append((s, 3))
        s += 3
    if ns - s == 4:
        macros.extend([(s, 2), (s + 2, 2)])
    elif ns - s > 0:
        macros.append((s, ns - s))
    return macros


def _build_bass(tpad, dbg=False, reps=0):
    import concourse.bass as bass
    import concourse.mybir as mybir
    import concourse.tile as tile
    from concourse import bacc

    f32 = mybir.dt.float32
    f32r = mybir.dt.float32r
    xdt = mybir.dt.bfloat16 if X_BF16 else f32r
    AF = mybir.ActivationFunctionType
    AX = mybir.AxisListType

    ns = tpad // SUB
    nt10 = tpad // G
    macros = _macro_split(ns)
    nmac = len(macros)

    nc = bacc.Bacc()
    x_h = nc.dram_tensor("x", [G, L, nt10, F], xdt, kind="ExternalInput")
    mk_h = nc.dram_tensor("masks", [2, tpad], f32, kind="ExternalInput")
    ag_h = nc.dram_tensor("aggw", [12, SUB, SUB], xdt, kind="ExternalInput")
    w1_h = nc.dram_tensor("w1t", [P, 6, H], f32r, kind="ExternalInput")
    w2_h = nc.dram_tensor("w2t", [P, 2, H], f32r, kind="ExternalInput")
    b1_h = nc.dram_tensor("b1s", [P, 2], f32, kind="ExternalInput")
    b2_h = nc.dram_tensor("b2s", [P, 2], f32, kind="ExternalInput")
    w3_h = nc.dram_tensor("w3t", [P, 2, NL], f32, kind="ExternalInput")
    b3_h = nc.dram_tensor("b3s", [NL, 1], f32, kind="ExternalInput")
    id_h = nc.dram_tensor("ident", [SUB, SUB], f32r, kind="ExternalInput")
    out_h = nc.dram_tensor("out", [2, NL], f32, kind="ExternalOutput")
    if dbg:
        dbg_hagg = nc.dram_tensor("dbg_hagg", [SUB, F], f32r, kind="ExternalOutput")
        dbg_h3 = nc.dram_tensor("dbg_h3", [P, 2, 3 * SUB], f32, kind="ExternalOutput")
        dbg_mask = nc.dram_tensor("dbg_mask", [P, 2, tpad], f32, kind="ExternalOutput")
        dbg_pool = nc.dram_tensor("dbg_pool", [P, 2, 2], f32, kind="ExternalOutput")
        nmac_ = len(_macro_split(tpad // SUB))
        dbg_acc = nc.dram_tensor("dbg_acc", [P, 2, 2, nmac_], f32, kind="ExternalOutput")
        dbg_h3b = nc.dram_tensor("dbg_h3b", [P, 2, 3 * SUB], f32, kind="ExternalOutput")

    with tile.TileContext(nc) as tc:
        with (
            tc.tile_pool(name="const", bufs=1) as const,
            tc.tile_pool(name="xp", bufs=6) as xp,
            tc.tile_pool(name="hp", bufs=2) as hp,
            tc.tile_pool(name="tp", bufs=2) as tp,
            tc.tile_pool(name="fin", bufs=2) as fin,
            tc.tile_pool(name="psA", bufs=2, space="PSUM") as psA,
            tc.tile_pool(name="psT", bufs=1, space="PSUM") as psT,
            tc.tile_pool(name="ps1", bufs=1, space="PSUM") as ps1,
            tc.tile_pool(name="ps2", bufs=1, space="PSUM") as ps2,
            tc.tile_pool(name="ps3", bufs=1, space="PSUM") as ps3,
        ):
            # ---- constants into SBUF ----
            ag_sb = const.tile([SUB, 12, SUB], xdt)
            nc.sync.dma_start(
                out=ag_sb,
                in_=bass.AP(ag_h, 0, [[SUB, SUB], [SUB * SUB, 12], [1, SUB]]),
            )
            w1_sb = const.tile([P, 6, H], f32r)
            nc.sync.dma_start(out=w1_sb, in_=w1_h[:, :, :])
            w2_sb = const.tile([P, 2, H], f32r)
            nc.sync.dma_start(out=w2_sb, in_=w2_h[:, :, :])
            b1_sb = const.tile([P, 2], f32)
            nc.sync.dma_start(out=b1_sb, in_=b1_h[:, :])
            b2_sb = const.tile([P, 2], f32)
            nc.sync.dma_start(out=b2_sb, in_=b2_h[:, :])
            w3_sb = const.tile([P, 2, NL], f32)
            nc.sync.dma_start(out=w3_sb, in_=w3_h[:, :, :])
            b3_sb = const.tile([NL, 1], f32)
            nc.sync.dma_start(out=b3_sb, in_=b3_h[:, :])
            id_sb = const.tile([SUB, SUB], f32r)
            nc.sync.dma_start(out=id_sb, in_=id_h[:, :])
            mk_sb = const.tile([P, 2, tpad], f32)
            for s in range(2):
                nc.gpsimd.dma_start(
                    out=mk_sb[:, s, :],
                    in_=bass.AP(mk_h, s * tpad, [[0, P], [1, tpad]]),
                )
            acc_sb = const.tile([P, 2, 2, nmac], f32)

            import contextlib
            rep_ctx = tc.For_i(0, reps, 1) if reps else contextlib.nullcontext()
            with rep_ctx:
                _emit_body(nc, tc, bass, mybir, tpad, macros, dbg,
                           locals())
    nc.compile()
    return nc


def _emit_body(nc, tc, bass, mybir, tpad, macros, dbg, env):
    f32 = mybir.dt.float32
    f32r = mybir.dt.float32r
    AF = mybir.ActivationFunctionType
    AX = mybir.AxisListType
    nt10 = tpad // G
    (const, xp, hp, tp, fin, psA, psT, ps1, ps2, ps3) = (
        env[k] for k in
        ("const", "xp", "hp", "tp", "fin", "psA", "psT", "ps1", "ps2", "ps3"))
    ag_sb, w1_sb, w2_sb, b1_sb, b2_sb, w3_sb, b3_sb, id_sb, mk_sb, acc_sb = (
        env[k] for k in ("ag_sb", "w1_sb", "w2_sb", "b1_sb", "b2_sb",
                         "w3_sb", "b3_sb", "id_sb", "mk_sb", "acc_sb"))
    x_h, out_h = env["x_h"], env["out_h"]
    if dbg:
        dbg_hagg, dbg_h3, dbg_mask, dbg_pool = (
            env[k] for k in ("dbg_hagg", "dbg_h3", "dbg_mask", "dbg_pool"))
        dbg_acc, dbg_h3b = env["dbg_acc"], env["dbg_h3b"]
    if True:
        if True:
            # ---- main loop over macro tiles ----
            for mi, (s0, msubs) in enumerate(macros):
                W = msubs * SUB
                haggT = tp.tile([P, 6, 3 * SUB], f32r, tag="haggT")
                for sl in range(msubs):
                    st = s0 + sl
                    x_sb = xp.tile([SUB, 12 * F],
                                   mybir.dt.bfloat16 if X_BF16 else f32r,
                                   tag="x")
                    nc.sync.dma_start(
                        out=x_sb,
                        in_=bass.AP(
                            x_h, 12 * st * F, [[nt10 * F, SUB], [1, 12 * F]]
                        ),
                    )
                    for fc in range(2):
                        agg_ps = psA.tile([SUB, FC], f32, tag="agg")
                        for i in range(12):
                            nc.tensor.matmul(
                                agg_ps,
                                lhsT=ag_sb[:, i, :],
                                rhs=x_sb[:, i * F + fc * FC:
                                         i * F + (fc + 1) * FC],
                                start=(i == 0),
                                stop=(i == 11),
                            )
                        hagg = hp.tile([SUB, FC], f32r, tag="hagg")
                        nc.scalar.copy(out=hagg, in_=agg_ps)
                        if dbg and st == 0:
                            nc.sync.dma_start(
                                out=bass.AP(dbg_hagg, fc * FC,
                                            [[F, SUB], [1, FC]]),
                                in_=hagg,
                            )
                        tr_ps = psT.tile([P, 3, SUB], f32r, tag="tr")
                        for j in range(3):
                            nc.tensor.transpose(
                                tr_ps[:, j, :],
                                hagg[:, j * P:(j + 1) * P],
                                id_sb,
                            )
                        nc.vector.tensor_copy(
                            out=haggT[:, fc * 3:(fc + 1) * 3,
                                      sl * SUB:(sl + 1) * SUB],
                            in_=tr_ps,
                        )
                # ---- w1 matmul + bias + relu ----
                mm1_ps = ps1.tile([P, 2, 512], f32, tag="mm1")
                for mh in range(2):
                    for kf in range(6):
                        nc.tensor.matmul(
                            mm1_ps[:, mh, :W],
                            lhsT=w1_sb[:, kf, mh * P:(mh + 1) * P],
                            rhs=haggT[:, kf, :W],
                            start=(kf == 0),
                            stop=(kf == 5),
                        )
                h2 = hp.tile([P, 2, 3 * SUB], f32r, tag="h2")
                for mh in range(2):
                    nc.scalar.activation(
                        out=h2[:, mh, :W],
                        in_=mm1_ps[:, mh, :W],
                        func=AF.Relu,
                        bias=b1_sb[:, mh:mh + 1],
                        scale=1.0,
                    )
                # ---- w2 matmul + bias + relu ----
                mm2_ps = ps2.tile([P, 2, 512], f32, tag="mm2")
                for mg in range(2):
                    for kh in range(2):
                        nc.tensor.matmul(
                            mm2_ps[:, mg, :W],
                            lhsT=w2_sb[:, kh, mg * P:(mg + 1) * P],
                            rhs=h2[:, kh, :W],
                            start=(kh == 0),
                            stop=(kh == 1),
                        )
                h3 = hp.tile([P, 2, 3 * SUB], f32, tag="h3")
                for mg in range(2):
                    nc.scalar.activation(
                        out=h3[:, mg, :W],
                        in_=mm2_ps[:, mg, :W],
                        func=AF.Relu,
                        bias=b2_sb[:, mg:mg + 1],
                        scale=1.0,
                    )
                if dbg and mi == 0:
                    nc.sync.dma_start(out=dbg_h3[:, :, :W], in_=h3[:, :, :W])
                if dbg and mi == 1:
                    nc.sync.dma_start(out=dbg_h3b[:, :, :W], in_=h3[:, :, :W])
                # ---- masked pooling (both segment slots) ----
                for s in range(2):
                    h3m = hp.tile([P, 2, 3 * SUB], f32, tag="h3m")
                    for mg in range(2):
                        nc.vector.tensor_mul(
                            out=h3m[:, mg, :W],
                            in0=h3[:, mg, :W],
                            in1=mk_sb[:, s, s0 * SUB:s0 * SUB + W],
                        )
                    nc.vector.reduce_sum(
                        out=acc_sb[:, s, :, mi],
                        in_=h3m[:, :, :W],
                        axis=AX.X,
                    )

            # ---- finale: reduce accumulators, classifier, write out ----
            if dbg:
                nc.sync.dma_start(out=dbg_mask[:, :, :], in_=mk_sb)
                nc.sync.dma_start(out=dbg_acc[:, :, :, :], in_=acc_sb)
            for s in range(2):
                pooled = fin.tile([P, 2], f32, tag="pooled")
                for kg in range(2):
                    nc.vector.reduce_sum(
                        out=pooled[:, kg:kg + 1],
                        in_=acc_sb[:, s, kg, :],
                        axis=AX.X,
                    )
                if dbg:
                    nc.sync.dma_start(
                        out=bass.AP(dbg_pool, s * 2, [[4, P], [1, 2]]),
                        in_=pooled,
                    )
                mm3_ps = ps3.tile([NL, 1], f32, tag="mm3")
                for kg in range(2):
                    nc.tensor.matmul(
                        mm3_ps,
                        lhsT=w3_sb[:, kg, :],
                        rhs=pooled[:, kg:kg + 1],
                        start=(kg == 0),
                        stop=(kg == 1),
                    )
                o_sb = fin.tile([NL, 1], f32, tag="osb")
                nc.scalar.add(out=o_sb, in_=mm3_ps, add=b3_sb)
                nc.sync.dma_start(
                    out=bass.AP(out_h, s * NL, [[1, NL]]),
                    in_=o_sb,
                )


def _xdt_np():
    if X_BF16:
        import ml_dtypes
        return ml_dtypes.bfloat16
    return np.float32


def _prep_shared(aggr_w, w1, b1, w2, b2, w3, b3):
    aggw = np.zeros((12, SUB, SUB), dtype=np.float32)
    for i in range(12):
        for gt in range(G):
            for l in range(L):
                aggw[i, gt * L + l, i * G + gt] = aggr_w[l]
    w1t = np.ascontiguousarray(
        w1.T.reshape(6, P, H).transpose(1, 0, 2)).astype(np.float32)
    w2t = np.ascontiguousarray(
        w2.T.reshape(2, P, H).transpose(1, 0, 2)).astype(np.float32)
    w3t = np.ascontiguousarray(
        w3.T.reshape(2, P, NL).transpose(1, 0, 2)).astype(np.float32)
    b1s = np.ascontiguousarray(b1.reshape(2, P).T).astype(np.float32)
    b2s = np.ascontiguousarray(b2.reshape(2, P).T).astype(np.float32)
    b3s = b3.reshape(NL, 1).astype(np.float32)
    ident = np.eye(SUB, dtype=np.float32)
    return {
        "aggw": aggw.astype(_xdt_np()), "w1t": w1t, "w2t": w2t,
        "b1s": b1s, "b2s": b2s, "w3t": w3t, "b3s": b3s, "ident": ident,
    }


def kernel(x, lengths, aggr_w, w1, b1, w2, b2, w3, b3):
    global LAST_RESULTS
    from concourse.bass_utils import run_bass_kernel_spmd

    x = np.asarray(x, dtype=np.float32)
    lens = np.asarray(lengths).astype(np.int64)
    aggr_w = np.asarray(aggr_w, dtype=np.float32)
    w1 = np.asarray(w1, dtype=np.float32)
    b1 = np.asarray(b1, dtype=np.float32)
    w2 = np.asarray(w2, dtype=np.float32)
    b2 = np.asarray(b2, dtype=np.float32)
    w3 = np.asarray(w3, dtype=np.float32)
    b3 = np.asarray(b3, dtype=np.float32)

    # pair longest with shortest to balance per-core work
    order = np.argsort(-lens, kind="stable")
    pairs = [(int(order[i]), int(order[B - 1 - i])) for i in range(NCORES)]
    psum_max = max(int(lens[a] + lens[b]) for a, b in pairs)
    tpad = max(SUB, ((psum_max + SUB - 1) // SUB) * SUB)
    nt10 = tpad // G

    if tpad not in _CACHE:
        _CACHE[tpad] = _build_bass(tpad)
    nc = _CACHE[tpad]

    shared = _prep_shared(aggr_w, w1, b1, w2, b2, w3, b3)
    in_maps = []
    for a, b in pairs:
        la, lb = int(lens[a]), int(lens[b])
        xt = np.zeros((L, tpad, F), dtype=np.float32)
        xt[:, :la] = x[a, :, :la]
        xt[:, la:la + lb] = x[b, :, :lb]
        # xq[gt, l, g, f] = xt[l, 10*g + gt, f]
        xq = np.ascontiguousarray(
            xt.reshape(L, nt10, G, F).transpose(2, 0, 1, 3)).astype(_xdt_np())
        masks = np.zeros((2, tpad), dtype=np.float32)
        masks[0, :la] = 1.0 / la
        masks[1, la:la + lb] = 1.0 / lb
        in_maps.append({"x": xq, "masks": masks, **shared})

    res = run_bass_kernel_spmd(nc, in_maps, core_ids=list(range(NCORES)))
    LAST_RESULTS = res

    out = np.zeros((B, NL), dtype=np.float32)
    for c, (a, b) in enumerate(pairs):
        out[a] = res.results[c]["out"][0]
        out[b] = res.results[c]["out"][1]
    return out
